# revision 20
# baseline (speedup 1.0000x reference)
"""Trainium2 Bass kernel for nn_CAUSETRModel (VQ codebook + TR decoder).

Sharding: data-parallel over batch B=16 across 8 NeuronCores (2 batch
elements per core).  Everything else (codebook, decoder weights) is
replicated.  Inside each core the two batch elements are concatenated
along the token axis (2 x 896-padded = 1792 "slots", 784 real tokens
each).

Device dataflow (per core), feature-major activations [d on partitions,
tokens on free]:
  1. VQ scoring S = feat @ cbn^T as three bf16 matmuls (hi/lo split of
     both operands) so the cosine argmax matches fp32 exactly; row argmax
     via vector.max/max_index; codebook row gather via gpsimd.dma_gather.
  2. dq^T via PE transpose -> residual stream tgt (f32).
  3. Self-attn, cross-attn (per batch element), FFN, 3 layernorms
     (stats via ones-vector matmul on PE, broadcast via gpsimd),
     all matmuls in bf16 with f32 PSUM accumulation.
  4. Heads (f1 / f2a / f2b / proj) and token-major outputs via PE
     transpose.
"""

import sys
import types
from contextlib import ExitStack

for _p in ("/opt/trn_rl_repo", "/root/.axon_site"):
    if _p not in sys.path:
        sys.path.insert(0, _p)

import numpy as np
import ml_dtypes

BF = ml_dtypes.bfloat16

# ---- problem shapes (hardcoded) ----
B, SEQ, D = 16, 784, 1024
RD, PD, NCODE, HID = 90, 2048, 2048, 2048
EPS = 1e-5
NCORES = 8
BPC = B // NCORES          # batch elements per core = 2
SP = 896                   # per-batch padded token span (7*128)
S = BPC * SP               # 1792 concat padded tokens per core
ST = S // 128              # 14 token tiles
KD = D // 128              # 8 d-chunks
HK = HID // 128            # 16 hidden chunks
P = 128

_PROG = {}


def _ensure_ntff_hook():
    """Inject antenv.axon_hooks (absent in this image) so
    run_bass_kernel_spmd(trace=True) can profile via the axon .so."""
    import antenv
    if "antenv.axon_hooks" in sys.modules:
        return
    mod = types.ModuleType("antenv.axon_hooks")
    mod._hook = None
    mod.set_axon_ntff_profile_hook = lambda h: setattr(mod, "_hook", h)
    mod.get_axon_ntff_profile_hook = lambda: mod._hook
    sys.modules["antenv.axon_hooks"] = mod
    antenv.axon_hooks = mod
    try:
        from trn_agent_boot.trn_boot import _ntff_profile_via_ctypes
        mod.set_axon_ntff_profile_hook(
            _ntff_profile_via_ctypes("/opt/axon/libaxon_pjrt.so"))
    except Exception:
        pass


def _chunks(total, cap):
    out = []
    o = 0
    while o < total:
        c = min(cap, total - o)
        out.append((o, c))
        o += c
    return out


def _build_program():
    import concourse.bass as bass
    import concourse.tile as tile
    from concourse import bacc, mybir
    from concourse.masks import make_identity

    f32 = mybir.dt.float32
    bf16 = mybir.dt.bfloat16
    i16 = mybir.dt.int16
    u16 = mybir.dt.uint16
    Alu = mybir.AluOpType
    Act = mybir.ActivationFunctionType

    nc = bacc.Bacc("TRN2", target_bir_lowering=False, debug=False,
                   num_devices=NCORES)

    def din(name, shape, dt=f32):
        return nc.dram_tensor(name, list(shape), dt, kind="ExternalInput").ap()

    def dout(name, shape, dt=f32):
        return nc.dram_tensor(name, list(shape), dt, kind="ExternalOutput").ap()

    # ---- DRAM parameters ----
    featT = din("featT", (D, S))                  # f32, zero-padded cols
    featT_h = din("featT_h", (D, S), bf16)
    featT_l = din("featT_l", (D, S), bf16)
    cbnT_h = din("cbnT_h", (D, NCODE), bf16)
    cbnT_l = din("cbnT_l", (D, NCODE), bf16)
    codebook = din("codebook", (NCODE, D))
    posT = din("posT", (D, SEQ), bf16)

    w_sa = {n: din(f"sa_{n}T", (D, D), bf16) for n in ("wq", "wk", "wv", "wo")}
    w_ca = {n: din(f"ca_{n}T", (D, D), bf16) for n in ("wq", "wk", "wv", "wo")}
    lin1T = din("lin1T", (D, HID), bf16)
    lin2T = din("lin2T", (HID, D), bf16)
    f1T = din("f1T", (D, P), bf16)                # cols 90.. zero
    f2aT = din("f2aT", (D, D), bf16)
    f2bT = din("f2bT", (D, P), bf16)
    phT = din("phT", (P, PD), bf16)               # row 90 = ph_b, rows 91.. zero

    b_sa = {n: din(f"sa_{n}", (D,)) for n in ("bq", "bk", "bv", "bo")}
    b_ca = {n: din(f"ca_{n}", (D,)) for n in ("bq", "bk", "bv", "bo")}
    lin1_b = din("lin1_b", (HID,))
    lin2_b = din("lin2_b", (D,))
    f2a_b = din("f2a_b", (D,))
    f1_b = din("f1_b", (P,))                      # padded to 128
    f2b_b = din("f2b_b", (P,))
    ln_par = {n: din(n, (D,)) for n in
              ("n1_g", "n1_b", "n2_g", "n2_b", "n3_g", "n3_b")}

    tr_out = dout("tr_out", (BPC, SEQ, RD))
    proj_out = dout("proj_out", (BPC, SEQ, PD))

    # feature-major DRAM views
    def fmv(ap, nk):  # (nk*128, F) -> [128, nk, F]
        return ap.rearrange("(k p) f -> p k f", p=P)

    def colv(ap):     # (nk*128,) -> [128, nk]
        return ap.rearrange("(k p) -> p k", p=P)

    with tile.TileContext(nc) as tc, ExitStack() as top:
        const = top.enter_context(tc.tile_pool(name="const", bufs=1))
        ident_f = const.tile([P, P], f32)
        make_identity(nc, ident_f)
        ident_b = const.tile([P, P], bf16)
        make_identity(nc, ident_b)
        ones_col = const.tile([P, 1], bf16)
        nc.vector.memset(ones_col, 1.0)

        # all biases / ln params resident (tiny)
        bias = top.enter_context(tc.tile_pool(name="bias", bufs=1))

        def load_col(ap, nk, label):
            t = bias.tile([P, nk], f32, name=f"bc_{label}", tag=f"bc_{label}")
            nc.sync.dma_start(t[:], colv(ap))
            return t

        b_sa_s = {n: load_col(a, KD, f"sa{n}") for n, a in b_sa.items()}
        b_ca_s = {n: load_col(a, KD, f"ca{n}") for n, a in b_ca.items()}
        lin1_b_s = load_col(lin1_b, HK, "l1b")
        lin2_b_s = load_col(lin2_b, KD, "l2b")
        f2a_b_s = load_col(f2a_b, KD, "f2ab")
        f1_b_s = load_col(f1_b, 1, "f1b")
        f2b_b_s = load_col(f2b_b, 1, "f2bb")
        ln_s = {n: load_col(a, KD, n) for n, a in ln_par.items()}

        mmp = top.enter_context(tc.tile_pool(name="mmp", bufs=4, space="PSUM"))
        tpp = top.enter_context(tc.tile_pool(name="tpp", bufs=2, space="PSUM"))

        # ================= Stage A: VQ =================
        with ExitStack() as sa_stack:
            idxp = sa_stack.enter_context(tc.tile_pool(name="idxp", bufs=1))
            idx32 = idxp.tile([P, ST], mybir.dt.uint32)
            nc.vector.memset(idx32[:], 0)

            with ExitStack() as sc_stack:
                sc = sc_stack.enter_context(tc.tile_pool(name="score", bufs=1))
                xh = sc.tile([P, KD, S], bf16)
                xl = sc.tile([P, KD, S], bf16)
                ch = sc.tile([P, KD, NCODE], bf16)
                cl = sc.tile([P, KD, NCODE], bf16)
                for k in range(KD):
                    for (n0, nl) in _chunks(NCODE, 512):
                        nc.sync.dma_start(ch[:, k, n0:n0 + nl],
                                          fmv(cbnT_h, KD)[:, k, n0:n0 + nl])
                        nc.sync.dma_start(cl[:, k, n0:n0 + nl],
                                          fmv(cbnT_l, KD)[:, k, n0:n0 + nl])
                    for (n0, nl) in _chunks(S, 448):
                        nc.sync.dma_start(xh[:, k, n0:n0 + nl],
                                          fmv(featT_h, KD)[:, k, n0:n0 + nl])
                        nc.sync.dma_start(xl[:, k, n0:n0 + nl],
                                          fmv(featT_l, KD)[:, k, n0:n0 + nl])

                spool = sc_stack.enter_context(tc.tile_pool(name="svq", bufs=2))
                m8p = sc_stack.enter_context(tc.tile_pool(name="m8", bufs=2))

                for t in range(ST):
                    s_t = spool.tile([P, NCODE], f32, tag="svq")
                    for (n0, nl) in _chunks(NCODE, 512):
                        ps = mmp.tile([P, 512], f32, tag="mm")
                        cnt = 0
                        for (a, c) in ((xh, ch), (xh, cl), (xl, ch)):
                            for k in range(KD):
                                nc.tensor.matmul(
                                    ps[:, :nl],
                                    lhsT=a[:, k, t * P:(t + 1) * P],
                                    rhs=c[:, k, n0:n0 + nl],
                                    start=(cnt == 0), stop=(cnt == 23))
                                cnt += 1
                        nc.vector.tensor_copy(s_t[:, n0:n0 + nl], ps[:, :nl])
                    mx = m8p.tile([P, 8], f32, tag="mx")
                    ix = m8p.tile([P, 8], u16, tag="ix")
                    nc.vector.max(mx, s_t)
                    nc.vector.max_index(ix, mx, s_t)
                    v = 16 if t in (6, 13) else P   # pad slots keep index 0
                    nc.vector.tensor_copy(idx32[:v, t:t + 1], ix[:v, 0:1])

            dqp = sa_stack.enter_context(tc.tile_pool(name="dq", bufs=1))
            dq = dqp.tile([P, ST, D], f32)
            for t in range(ST):
                nc.gpsimd.indirect_dma_start(
                    out=dq[:, t, :], out_offset=None, in_=codebook,
                    in_offset=bass.IndirectOffsetOnAxis(
                        ap=idx32[:, t:t + 1], axis=0))

            # persistent residual stream (f32, feature-major, right side
            # so it doesn't interleave with the left-side stage stack)
            resid = top.enter_context(
                tc.tile_pool(name="resid", bufs=1, side="right"))
            tgt = resid.tile([P, KD, S], f32)

            # dq^T -> tgt (f32)
            for t in range(ST):
                for k in range(KD):
                    pt = mmp.tile([P, P], f32, tag="mm", name="ptf")
                    nc.tensor.transpose(pt, dq[:, t, k * P:(k + 1) * P], ident_f)
                    nc.vector.tensor_copy(tgt[:, k, t * P:(t + 1) * P], pt)

        # ---------- helpers ----------
        def layer_norm(g_col, b_col, x_out=None):
            """Per-448-token-chunk pipelined layernorm over d (partitions).
            Stats via ones-vector matmul on PE, rstd chain on [1,448] rows,
            broadcast on gpsimd, apply on DVE.  x_out: optional bf16 shadow."""
            with ExitStack() as ln_stack:
                rows = ln_stack.enter_context(tc.tile_pool(name="lnrows", bufs=2))
                cast = ln_stack.enter_context(tc.tile_pool(name="lncast", bufs=3))
                stp = ln_stack.enter_context(
                    tc.tile_pool(name="lnps", bufs=1, space="PSUM"))
                bc = ln_stack.enter_context(tc.tile_pool(name="lnbc", bufs=2))
                for (n0, nl) in _chunks(S, 448):
                    ps_s = stp.tile([1, 448], f32, tag="st_s", name="ps_s")
                    ps_q = stp.tile([1, 448], f32, tag="st_q", name="ps_q")
                    for k in range(KD):
                        xb = cast.tile([P, 448], bf16, tag="xb", name="xb")
                        nc.vector.tensor_copy(xb[:, :nl], tgt[:, k, n0:n0 + nl])
                        sq = cast.tile([P, 448], bf16, tag="sq", name="sq")
                        nc.scalar.activation(sq[:, :nl], xb[:, :nl], Act.Square)
                        nc.tensor.matmul(ps_s[:, :nl], lhsT=ones_col,
                                         rhs=xb[:, :nl],
                                         start=(k == 0), stop=(k == KD - 1))
                        nc.tensor.matmul(ps_q[:, :nl], lhsT=ones_col,
                                         rhs=sq[:, :nl],
                                         start=(k == 0), stop=(k == KD - 1))
                    mu = rows.tile([1, 448], f32, tag="mu", name="mu")
                    nc.vector.tensor_scalar_mul(mu[:, :nl], ps_s[:, :nl], 1.0 / D)
                    rst = rows.tile([1, 448], f32, tag="rst", name="rst")
                    # rst = meansq - mu^2 + eps -> sqrt -> reciprocal
                    nc.vector.tensor_scalar_mul(rst[:, :nl], ps_q[:, :nl], 1.0 / D)
                    msq = rows.tile([1, 448], f32, tag="msq", name="msq")
                    nc.vector.tensor_mul(msq[:, :nl], mu[:, :nl], mu[:, :nl])
                    nc.vector.tensor_tensor(rst[:, :nl], rst[:, :nl], msq[:, :nl],
                                            Alu.subtract)
                    nc.vector.tensor_scalar_add(rst[:, :nl], rst[:, :nl], EPS)
                    nc.scalar.activation(rst[:, :nl], rst[:, :nl], Act.Sqrt)
                    nc.vector.reciprocal(rst[:, :nl], rst[:, :nl])
                    mu_bc = bc.tile([P, 448], f32, tag="mubc", name="mu_bc")
                    nc.gpsimd.partition_broadcast(mu_bc[:, :nl], mu[:, :nl])
                    rs_bc = bc.tile([P, 448], f32, tag="rsbc", name="rs_bc")
                    nc.gpsimd.partition_broadcast(rs_bc[:, :nl], rst[:, :nl])
                    for k in range(KD):
                        xc = tgt[:, k, n0:n0 + nl]
                        # spread across engines: sub on gpsimd, (x*g)*rstd
                        # on DVE, +b on ACT, shadow cast on DVE
                        nc.gpsimd.tensor_tensor(xc, xc, mu_bc[:, :nl],
                                                Alu.subtract)
                        nc.vector.scalar_tensor_tensor(
                            xc, xc, g_col[:, k:k + 1], rs_bc[:, :nl],
                            Alu.mult, Alu.mult)
                        nc.scalar.activation(xc, xc, Act.Identity,
                                             bias=b_col[:, k:k + 1])
                        if x_out is not None:
                            nc.vector.tensor_copy(x_out[:, k, n0:n0 + nl], xc)

        def load_w(pool, dram_ap, nk, tag):
            t = pool.tile([P, nk, dram_ap.shape[-1]], bf16,
                          name=f"w_{tag}", tag=tag)
            nc.sync.dma_start(t[:], fmv(dram_ap, nk))
            return t

        def attention(wd, bd, v_from_tgt):
            """One MHA block + residual add into tgt.
            v_from_tgt: True -> k/v input is tgt (self-attn, k=q input);
                        False -> k/v input is feat (cross-attn)."""
            with ExitStack() as att:
                qkp = att.enter_context(tc.tile_pool(name="qk", bufs=1))
                qk_b = [qkp.tile([P, KD, SP], bf16, name=f"qkb{b}",
                                 tag=f"qkb{b}") for b in range(BPC)]
                with ExitStack() as post:
                    pospool = post.enter_context(
                        tc.tile_pool(name="pos", bufs=1))
                    pos_s = pospool.tile([P, KD, SEQ], bf16)
                    nc.sync.dma_start(pos_s[:], fmv(posT, KD))
                    for b in range(BPC):
                        off = b * SP
                        for k in range(KD):
                            nc.vector.tensor_tensor(
                                qk_b[b][:, k, :SEQ], tgt[:, k, off:off + SEQ],
                                pos_s[:, k, :], Alu.add)
                            nc.vector.memset(qk_b[b][:, k, SEQ:], 0.0)

                for b in range(BPC):
                    off = b * SP
                    with ExitStack() as batt:
                        bufp = batt.enter_context(
                            tc.tile_pool(name="abuf", bufs=1))
                        qt = bufp.tile([P, KD, SP], bf16)
                        kt = bufp.tile([P, KD, SEQ], bf16)
                        vt = bufp.tile([P, SP // P, D], bf16)
                        at = bufp.tile([P, SP // P, SP], bf16)
                        ot = bufp.tile([P, KD, SEQ], bf16)

                        # k/v source chunks [128, SP] per d-chunk
                        srcp = batt.enter_context(
                            tc.tile_pool(name="kvsrc", bufs=1))
                        if v_from_tgt:
                            kv_src = qk_b[b]  # unused marker
                            vsrc = srcp.tile([P, KD, SP], bf16)
                            for k in range(KD):
                                nc.vector.tensor_copy(
                                    vsrc[:, k, :], tgt[:, k, off:off + SP])
                        else:
                            vsrc = srcp.tile([P, KD, SP], bf16)
                            nc.sync.dma_start(
                                vsrc[:],
                                fmv(featT_h, KD)[:, :, off:off + SP])

                        with ExitStack() as wst:
                            wp = wst.enter_context(
                                tc.tile_pool(name="wqk", bufs=1))
                            wq_s = load_w(wp, wd["wq"], KD, "wq")
                            wk_s = load_w(wp, wd["wk"], KD, "wk")
                            # Q^T [128, KD, SP]
                            for (n0, nl) in _chunks(SP, 448):
                                for e in range(KD):
                                    ps = mmp.tile([P, 512], f32, tag="mm")
                                    for k in range(KD):
                                        nc.tensor.matmul(
                                            ps[:, :nl],
                                            lhsT=wq_s[:, k, e * P:(e + 1) * P],
                                            rhs=qk_b[b][:, k, n0:n0 + nl],
                                            start=(k == 0), stop=(k == KD - 1))
                                    nc.vector.tensor_scalar_add(
                                        qt[:, e, n0:n0 + nl], ps[:, :nl],
                                        bd["bq"][:, e:e + 1])
                            # K^T [128, KD, SEQ]
                            k_rhs = qk_b[b] if v_from_tgt else vsrc
                            for (n0, nl) in _chunks(SEQ, 392):
                                for e in range(KD):
                                    ps = mmp.tile([P, 512], f32, tag="mm")
                                    for k in range(KD):
                                        nc.tensor.matmul(
                                            ps[:, :nl],
                                            lhsT=wk_s[:, k, e * P:(e + 1) * P],
                                            rhs=k_rhs[:, k, n0:n0 + nl],
                                            start=(k == 0), stop=(k == KD - 1))
                                    nc.vector.tensor_scalar_add(
                                        kt[:, e, n0:n0 + nl], ps[:, :nl],
                                        bd["bk"][:, e:e + 1])
                        with ExitStack() as wst:
                            wp = wst.enter_context(
                                tc.tile_pool(name="wv", bufs=1))
                            wv_s = load_w(wp, wd["wv"], KD, "wv")
                            # V token-major [128, 7, D]
                            for (n0, nl) in _chunks(D, 512):
                                for t in range(SP // P):
                                    ps = mmp.tile([P, 512], f32, tag="mm")
                                    for k in range(KD):
                                        nc.tensor.matmul(
                                            ps[:, :nl],
                                            lhsT=vsrc[:, k, t * P:(t + 1) * P],
                                            rhs=wv_s[:, k, n0:n0 + nl],
                                            start=(k == 0), stop=(k == KD - 1))
                                    nc.vector.tensor_copy(
                                        vt[:, t, n0:n0 + nl], ps[:, :nl])

                        # attention core
                        smp = batt.enter_context(tc.tile_pool(name="sm", bufs=2))
                        smallp = batt.enter_context(
                            tc.tile_pool(name="smsc", bufs=2))
                        for qi in range(SP // P):
                            s_ps = []
                            for (n0, nl) in _chunks(SEQ, 392):
                                ps = mmp.tile([P, 512], f32, tag="mm")
                                for e in range(KD):
                                    nc.tensor.matmul(
                                        ps[:, :nl],
                                        lhsT=qt[:, e, qi * P:(qi + 1) * P],
                                        rhs=kt[:, e, n0:n0 + nl],
                                        start=(e == 0), stop=(e == KD - 1))
                                s_ps.append((ps, n0, nl))
                            # rowmax + exp straight from PSUM
                            mxs = []
                            for i, (ps, n0, nl) in enumerate(s_ps):
                                mx = smallp.tile([P, 1], f32, tag=f"mx{i}",
                                                 name=f"mx{i}")
                                nc.vector.tensor_reduce(
                                    mx, ps[:, :nl], mybir.AxisListType.X,
                                    Alu.max)
                                mxs.append(mx)
                            nb = smallp.tile([P, 1], f32, tag="nb", name="nb")
                            nc.vector.tensor_tensor(nb, mxs[0], mxs[1], Alu.max)
                            nc.vector.tensor_scalar_mul(nb, nb, -1.0 / 32.0)
                            a_t = smp.tile([P, SP], bf16, tag="arow")
                            rss = []
                            for i, (ps, n0, nl) in enumerate(s_ps):
                                rs = smallp.tile([P, 1], f32, tag=f"rs{i}",
                                                 name=f"rs{i}")
                                nc.scalar.activation(a_t[:, n0:n0 + nl],
                                                     ps[:, :nl], Act.Exp,
                                                     bias=nb, scale=1.0 / 32.0,
                                                     accum_out=rs)
                                rss.append(rs)
                            nc.vector.memset(a_t[:, SEQ:], 0.0)
                            ri = smallp.tile([P, 1], f32, tag="ri", name="ri")
                            nc.vector.tensor_tensor(ri, rss[0], rss[1], Alu.add)
                            nc.vector.reciprocal(ri, ri)
                            nc.vector.tensor_scalar_mul(a_t[:, :SEQ],
                                                        a_t[:, :SEQ], ri)
                            for kc in range(SP // P):
                                pt = tpp.tile([P, P], bf16, tag="tpb")
                                nc.tensor.transpose(
                                    pt, a_t[:, kc * P:(kc + 1) * P], ident_b)
                                nc.vector.tensor_copy(
                                    at[:, kc, qi * P:(qi + 1) * P], pt)
                        # O^T = V^T A^T (+bv) [128, KD, SEQ]
                        for e in range(KD):
                            for (n0, nl) in _chunks(SEQ, 392):
                                ps = mmp.tile([P, 512], f32, tag="mm")
                                for kc in range(SP // P):
                                    nc.tensor.matmul(
                                        ps[:, :nl],
                                        lhsT=vt[:, kc, e * P:(e + 1) * P],
                                        rhs=at[:, kc, n0:n0 + nl],
                                        start=(kc == 0), stop=(kc == SP // P - 1))
                                nc.vector.tensor_scalar_add(
                                    ot[:, e, n0:n0 + nl], ps[:, :nl],
                                    bd["bv"][:, e:e + 1])
                        # out-proj + bias + residual into tgt
                        with ExitStack() as wst:
                            wp = wst.enter_context(
                                tc.tile_pool(name="wo", bufs=1))
                            wo_s = load_w(wp, wd["wo"], KD, "wo")
                            for (n0, nl) in _chunks(SEQ, 392):
                                for o in range(KD):
                                    ps = mmp.tile([P, 512], f32, tag="mm")
                                    for e in range(KD):
                                        nc.tensor.matmul(
                                            ps[:, :nl],
                                            lhsT=wo_s[:, e, o * P:(o + 1) * P],
                                            rhs=ot[:, e, n0:n0 + nl],
                                            start=(e == 0), stop=(e == KD - 1))
                                    nc.vector.scalar_tensor_tensor(
                                        tgt[:, o, off + n0:off + n0 + nl],
                                        ps[:, :nl], bd["bo"][:, o:o + 1],
                                        tgt[:, o, off + n0:off + n0 + nl],
                                        Alu.add, Alu.add)

        # ================= decoder =================
        attention(w_sa, b_sa_s, v_from_tgt=True)
        layer_norm(ln_s["n1_g"], ln_s["n1_b"])
        attention(w_ca, b_ca_s, v_from_tgt=False)
        # x_h = bf16(tgt2), written chunk-wise by LN2
        xhp = top.enter_context(tc.tile_pool(name="xh2", bufs=1))
        x_h = xhp.tile([P, KD, S], bf16)
        layer_norm(ln_s["n2_g"], ln_s["n2_b"], x_out=x_h)

        # ---- FFN ----
        with ExitStack() as ffn:
            hp = ffn.enter_context(tc.tile_pool(name="hbuf", bufs=1))
            h_s = hp.tile([P, HK, S], bf16)
            with ExitStack() as w2l:
                w2p1 = w2l.enter_context(tc.tile_pool(name="w2a", bufs=1))
                w2a = w2p1.tile([P, KD, D], bf16)
                nc.sync.dma_start(w2a[:], fmv(lin2T, HK)[:, 0:KD, :])
                with ExitStack() as w1l:
                    w1p = w1l.enter_context(tc.tile_pool(name="w1", bufs=1))
                    w1 = load_w(w1p, lin1T, KD, "w1")
                    for (n0, nl) in _chunks(S, 448):
                        for ht in range(HK):
                            ps = mmp.tile([P, 512], f32, tag="mm")
                            for k in range(KD):
                                nc.tensor.matmul(
                                    ps[:, :nl],
                                    lhsT=w1[:, k, ht * P:(ht + 1) * P],
                                    rhs=x_h[:, k, n0:n0 + nl],
                                    start=(k == 0), stop=(k == KD - 1))
                            nc.scalar.activation(h_s[:, ht, n0:n0 + nl],
                                                 ps[:, :nl], Act.Relu,
                                                 bias=lin1_b_s[:, ht:ht + 1])
                w2p2 = w2l.enter_context(tc.tile_pool(name="w2b", bufs=1))
                w2b = w2p2.tile([P, KD, D], bf16)
                nc.sync.dma_start(w2b[:], fmv(lin2T, HK)[:, KD:HK, :])
                for (n0, nl) in _chunks(S, 448):
                    for o in range(KD):
                        ps = mmp.tile([P, 512], f32, tag="mm")
                        for hk in range(HK):
                            w2t = w2a if hk < KD else w2b
                            nc.tensor.matmul(
                                ps[:, :nl],
                                lhsT=w2t[:, hk % KD, o * P:(o + 1) * P],
                                rhs=h_s[:, hk, n0:n0 + nl],
                                start=(hk == 0), stop=(hk == HK - 1))
                        nc.vector.scalar_tensor_tensor(
                            tgt[:, o, n0:n0 + nl], ps[:, :nl],
                            lin2_b_s[:, o:o + 1], tgt[:, o, n0:n0 + nl],
                            Alu.add, Alu.add)

        layer_norm(ln_s["n3_g"], ln_s["n3_b"])

        # tgt += feat (stream f32 chunks)
        with ExitStack() as fr:
            fp = fr.enter_context(tc.tile_pool(name="fres", bufs=2))
            for k in range(KD):
                ft = fp.tile([P, S], f32, tag="ft")
                nc.sync.dma_start(ft[:], fmv(featT, KD)[:, k, :])
                nc.vector.tensor_tensor(tgt[:, k, :], tgt[:, k, :], ft[:],
                                        Alu.add)
        for k in range(KD):
            nc.vector.tensor_copy(x_h[:, k, :], tgt[:, k, :])

        # ---- heads ----
        with ExitStack() as hd:
            hdp = hd.enter_context(tc.tile_pool(name="heads", bufs=1))
            wpool = hd.enter_context(tc.tile_pool(name="headw", bufs=1))
            outp = hd.enter_context(tc.tile_pool(name="outs", bufs=3))
            b1T = hdp.tile([P, S], f32)
            f1_s = load_w(wpool, f1T, KD, "f1")
            for (n0, nl) in _chunks(S, 448):
                ps = mmp.tile([P, 512], f32, tag="mm")
                for k in range(KD):
                    nc.tensor.matmul(ps[:, :nl], lhsT=f1_s[:, k, :],
                                     rhs=x_h[:, k, n0:n0 + nl],
                                     start=(k == 0), stop=(k == KD - 1))
                nc.vector.tensor_scalar_add(b1T[:, n0:n0 + nl], ps[:, :nl],
                                            f1_b_s[:, 0:1])
            h2 = hdp.tile([P, KD, S], bf16)
            f2a_s = load_w(wpool, f2aT, KD, "f2a")
            for (n0, nl) in _chunks(S, 448):
                for e in range(KD):
                    ps = mmp.tile([P, 512], f32, tag="mm")
                    for k in range(KD):
                        nc.tensor.matmul(
                            ps[:, :nl], lhsT=f2a_s[:, k, e * P:(e + 1) * P],
                            rhs=x_h[:, k, n0:n0 + nl],
                            start=(k == 0), stop=(k == KD - 1))
                    nc.scalar.activation(h2[:, e, n0:n0 + nl], ps[:, :nl],
                                         Act.Relu, bias=f2a_b_s[:, e:e + 1])
            trT = hdp.tile([P, S], f32)
            f2b_s = load_w(wpool, f2bT, KD, "f2b")
            for (n0, nl) in _chunks(S, 448):
                ps = mmp.tile([P, 512], f32, tag="mm")
                for e in range(KD):
                    nc.tensor.matmul(ps[:, :nl], lhsT=f2b_s[:, e, :],
                                     rhs=h2[:, e, n0:n0 + nl],
                                     start=(e == 0), stop=(e == KD - 1))
                nc.vector.scalar_tensor_tensor(
                    trT[:, n0:n0 + nl], ps[:, :nl], f2b_b_s[:, 0:1],
                    b1T[:, n0:n0 + nl], Alu.add, Alu.add)

            # tr output (token-major via PE transpose)
            for t in range(ST):
                b, qi = divmod(t, SP // P)
                v = min(P, SEQ - qi * P)
                if v <= 0:
                    continue
                pt = mmp.tile([P, P], f32, tag="mm", name="ptf")
                nc.tensor.transpose(pt, trT[:, t * P:(t + 1) * P], ident_f)
                stg = outp.tile([P, RD], f32, tag="trstg")
                nc.vector.tensor_copy(stg[:], pt[:, :RD])
                nc.sync.dma_start(tr_out[b, qi * P:qi * P + v, :], stg[:v, :])

            # proj head
            trh = hdp.tile([P, S], bf16)
            nc.vector.tensor_copy(trh[:], trT[:])
            nc.vector.memset(trh[96:97, :], 1.0)
            ph_s = wpool.tile([P, PD], bf16, tag="ph")
            nc.sync.dma_start(ph_s[:], phT[:])
            for t in range(ST):
                b, qi = divmod(t, SP // P)
                v = min(P, SEQ - qi * P)
                if v <= 0:
                    continue
                for (n0, nl) in _chunks(PD, 512):
                    ps = mmp.tile([P, 512], f32, tag="mm")
                    nc.tensor.matmul(ps[:, :nl],
                                     lhsT=trh[:, t * P:(t + 1) * P],
                                     rhs=ph_s[:, n0:n0 + nl],
                                     start=True, stop=True)
                    stg = outp.tile([P, 512], f32, tag="pstg")
                    nc.vector.tensor_copy(stg[:, :nl], ps[:, :nl])
                    nc.sync.dma_start(
                        proj_out[b, qi * P:qi * P + v, n0:n0 + nl],
                        stg[:v, :nl])

    nc.compile()
    return nc


def _get_program():
    if "nc" not in _PROG:
        _PROG["nc"] = _build_program()
    return _PROG["nc"]


def _split_bf16(x):
    hi = x.astype(BF)
    lo = (x - hi.astype(np.float32)).astype(BF)
    return hi, lo


def _prep_host(inputs):
    """Build the per-core input maps (host-side prep/transposes)."""
    f = lambda k: np.ascontiguousarray(np.asarray(inputs[k], dtype=np.float32))
    feat = f("feat")                     # (16, 784, 1024)
    cb = f("codebook")                   # (2048, 1024)
    cbn = cb / np.linalg.norm(cb, axis=1, keepdims=True)
    cbnT = np.ascontiguousarray(cbn.T)   # (1024, 2048)
    cbnT_h, cbnT_l = _split_bf16(cbnT)

    posT = np.ascontiguousarray(f("query_pos").T).astype(BF)   # (1024, 784)

    def wT(w):
        return np.ascontiguousarray(np.asarray(w, np.float32).T).astype(BF)

    common = {
        "cbnT_h": cbnT_h, "cbnT_l": cbnT_l, "codebook": cb, "posT": posT,
        "lin1T": wT(inputs["lin1_w"]), "lin2T": wT(inputs["lin2_w"]),
        "f2aT": wT(inputs["f2a_w"]),
        "lin1_b": f("lin1_b"), "lin2_b": f("lin2_b"), "f2a_b": f("f2a_b"),
    }
    for pre in ("sa", "ca"):
        wi = f(pre + "_wi")
        bi = f(pre + "_bi")
        common[pre + "_wqT"] = wT(wi[:D])
        common[pre + "_wkT"] = wT(wi[D:2 * D])
        common[pre + "_wvT"] = wT(wi[2 * D:])
        common[pre + "_woT"] = wT(inputs[pre + "_wo"])
        common[pre + "_bq"] = bi[:D].copy()
        common[pre + "_bk"] = bi[D:2 * D].copy()
        common[pre + "_bv"] = bi[2 * D:].copy()
        common[pre + "_bo"] = f(pre + "_bo")
    # padded heads
    f1Tp = np.zeros((D, P), np.float32)
    f1Tp[:, :RD] = f("f1_w").T
    common["f1T"] = f1Tp.astype(BF)
    f2bTp = np.zeros((D, P), np.float32)
    f2bTp[:, :RD] = f("f2b_w").T
    common["f2bT"] = f2bTp.astype(BF)
    phTp = np.zeros((P, PD), np.float32)
    phTp[:RD] = f("ph_w").T
    phTp[96] = f("ph_b")
    common["phT"] = phTp.astype(BF)
    f1b = np.zeros((P,), np.float32)
    f1b[:RD] = f("f1_b")
    common["f1_b"] = f1b
    f2bb = np.zeros((P,), np.float32)
    f2bb[:RD] = f("f2b_b")
    common["f2b_b"] = f2bb
    for n in ("n1_g", "n1_b", "n2_g", "n2_b", "n3_g", "n3_b"):
        common[n] = f(n)

    in_maps = []
    for c in range(NCORES):
        fc = feat[c * BPC:(c + 1) * BPC]          # (2, 784, 1024)
        featT_c = np.zeros((D, S), np.float32)
        for b in range(BPC):
            featT_c[:, b * SP:b * SP + SEQ] = fc[b].T
        fh, fl = _split_bf16(featT_c)
        m = dict(common)
        m["featT"] = featT_c
        m["featT_h"] = fh
        m["featT_l"] = fl
        in_maps.append(m)
    return in_maps


def kernel(**inputs):
    _ensure_ntff_hook()
    from concourse.bass_utils import run_bass_kernel_spmd

    nc = _get_program()
    in_maps = _prep_host(inputs)
    res = run_bass_kernel_spmd(nc, in_maps, core_ids=list(range(NCORES)))
    tr = np.concatenate([res.results[c]["tr_out"] for c in range(NCORES)], 0)
    proj = np.concatenate([res.results[c]["proj_out"] for c in range(NCORES)], 0)
    return tr.astype(np.float32), proj.astype(np.float32)


# revision 21
# speedup vs baseline: 1.0598x; 1.0598x over previous
"""Trainium2 Bass kernel for nn_CAUSETRModel (VQ codebook + TR decoder).

Sharding: data-parallel over batch B=16 across 8 NeuronCores (2 batch
elements per core).  Everything else (codebook, decoder weights) is
replicated.  Inside each core the two batch elements are concatenated
along the token axis (2 x 896-padded = 1792 "slots", 784 real tokens
each).

Device dataflow (per core), feature-major activations [d on partitions,
tokens on free]:
  1. VQ scoring S = feat @ cbn^T as three bf16 matmuls (hi/lo split of
     both operands) so the cosine argmax matches fp32 exactly; row argmax
     via vector.max/max_index; codebook row gather via gpsimd.dma_gather.
  2. dq^T via PE transpose -> residual stream tgt (f32).
  3. Self-attn, cross-attn (per batch element), FFN, 3 layernorms
     (stats via ones-vector matmul on PE, broadcast via gpsimd),
     all matmuls in bf16 with f32 PSUM accumulation.
  4. Heads (f1 / f2a / f2b / proj) and token-major outputs via PE
     transpose.
"""

import sys
import types
from contextlib import ExitStack

for _p in ("/opt/trn_rl_repo", "/root/.axon_site"):
    if _p not in sys.path:
        sys.path.insert(0, _p)

import numpy as np
import ml_dtypes

BF = ml_dtypes.bfloat16

# ---- problem shapes (hardcoded) ----
B, SEQ, D = 16, 784, 1024
RD, PD, NCODE, HID = 90, 2048, 2048, 2048
EPS = 1e-5
NCORES = 8
BPC = B // NCORES          # batch elements per core = 2
SP = 896                   # per-batch padded token span (7*128)
S = BPC * SP               # 1792 concat padded tokens per core
ST = S // 128              # 14 token tiles
KD = D // 128              # 8 d-chunks
HK = HID // 128            # 16 hidden chunks
P = 128

_PROG = {}


def _ensure_ntff_hook():
    """Inject antenv.axon_hooks (absent in this image) so
    run_bass_kernel_spmd(trace=True) can profile via the axon .so."""
    import antenv
    if "antenv.axon_hooks" in sys.modules:
        return
    mod = types.ModuleType("antenv.axon_hooks")
    mod._hook = None
    mod.set_axon_ntff_profile_hook = lambda h: setattr(mod, "_hook", h)
    mod.get_axon_ntff_profile_hook = lambda: mod._hook
    sys.modules["antenv.axon_hooks"] = mod
    antenv.axon_hooks = mod
    try:
        from trn_agent_boot.trn_boot import _ntff_profile_via_ctypes
        mod.set_axon_ntff_profile_hook(
            _ntff_profile_via_ctypes("/opt/axon/libaxon_pjrt.so"))
    except Exception:
        pass


def _chunks(total, cap):
    out = []
    o = 0
    while o < total:
        c = min(cap, total - o)
        out.append((o, c))
        o += c
    return out


def _build_program():
    import concourse.bass as bass
    import concourse.tile as tile
    from concourse import bacc, mybir
    from concourse.masks import make_identity

    f32 = mybir.dt.float32
    bf16 = mybir.dt.bfloat16
    i16 = mybir.dt.int16
    u16 = mybir.dt.uint16
    Alu = mybir.AluOpType
    Act = mybir.ActivationFunctionType

    nc = bacc.Bacc("TRN2", target_bir_lowering=False, debug=False,
                   num_devices=NCORES)

    def din(name, shape, dt=f32):
        return nc.dram_tensor(name, list(shape), dt, kind="ExternalInput").ap()

    def dout(name, shape, dt=f32):
        return nc.dram_tensor(name, list(shape), dt, kind="ExternalOutput").ap()

    # ---- DRAM parameters ----
    featT = din("featT", (D, S))                  # f32, zero-padded cols
    featT_h = din("featT_h", (D, S), bf16)
    featT_l = din("featT_l", (D, S), bf16)
    cbnT_h = din("cbnT_h", (D, NCODE), bf16)
    cbnT_l = din("cbnT_l", (D, NCODE), bf16)
    codebook = din("codebook", (NCODE, D))
    posT = din("posT", (D, SEQ), bf16)

    w_sa = {n: din(f"sa_{n}T", (D, D), bf16) for n in ("wq", "wk", "wv", "wo")}
    w_ca = {n: din(f"ca_{n}T", (D, D), bf16) for n in ("wq", "wk", "wv", "wo")}
    lin1T = din("lin1T", (D, HID), bf16)
    lin2T = din("lin2T", (HID, D), bf16)
    f1T = din("f1T", (D, P), bf16)                # cols 90.. zero
    f2aT = din("f2aT", (D, D), bf16)
    f2bT = din("f2bT", (D, P), bf16)
    phT = din("phT", (P, PD), bf16)               # row 90 = ph_b, rows 91.. zero

    b_sa = {n: din(f"sa_{n}", (D,)) for n in ("bq", "bk", "bv", "bo")}
    b_ca = {n: din(f"ca_{n}", (D,)) for n in ("bq", "bk", "bv", "bo")}
    lin1_b = din("lin1_b", (HID,))
    lin2_b = din("lin2_b", (D,))
    f2a_b = din("f2a_b", (D,))
    f1_b = din("f1_b", (P,))                      # padded to 128
    f2b_b = din("f2b_b", (P,))
    ln_par = {n: din(n, (D,)) for n in
              ("n1_g", "n1_b", "n2_g", "n2_b", "n3_g", "n3_b")}

    tr_out = dout("tr_out", (BPC, SEQ, RD))
    proj_out = dout("proj_out", (BPC, SEQ, PD))

    # feature-major DRAM views
    def fmv(ap, nk):  # (nk*128, F) -> [128, nk, F]
        return ap.rearrange("(k p) f -> p k f", p=P)

    def colv(ap):     # (nk*128,) -> [128, nk]
        return ap.rearrange("(k p) -> p k", p=P)

    with tile.TileContext(nc) as tc, ExitStack() as top:
        const = top.enter_context(tc.tile_pool(name="const", bufs=1))
        ident_f = const.tile([P, P], f32)
        make_identity(nc, ident_f)
        ident_b = const.tile([P, P], bf16)
        make_identity(nc, ident_b)
        ones_col = const.tile([P, 1], bf16)
        nc.vector.memset(ones_col, 1.0)

        # all biases / ln params resident (tiny)
        bias = top.enter_context(tc.tile_pool(name="bias", bufs=1))

        def load_col(ap, nk, label):
            t = bias.tile([P, nk], f32, name=f"bc_{label}", tag=f"bc_{label}")
            nc.sync.dma_start(t[:], colv(ap))
            return t

        b_sa_s = {n: load_col(a, KD, f"sa{n}") for n, a in b_sa.items()}
        b_ca_s = {n: load_col(a, KD, f"ca{n}") for n, a in b_ca.items()}
        lin1_b_s = load_col(lin1_b, HK, "l1b")
        lin2_b_s = load_col(lin2_b, KD, "l2b")
        f2a_b_s = load_col(f2a_b, KD, "f2ab")
        f1_b_s = load_col(f1_b, 1, "f1b")
        f2b_b_s = load_col(f2b_b, 1, "f2bb")
        ln_s = {n: load_col(a, KD, n) for n, a in ln_par.items()}

        mmp = top.enter_context(tc.tile_pool(name="mmp", bufs=4, space="PSUM"))
        tpp = top.enter_context(tc.tile_pool(name="tpp", bufs=2, space="PSUM"))

        # ================= Stage A: VQ =================
        with ExitStack() as sa_stack:
            idxp = sa_stack.enter_context(tc.tile_pool(name="idxp", bufs=1))
            idx32 = idxp.tile([P, ST], mybir.dt.uint32)
            nc.vector.memset(idx32[:], 0)

            with ExitStack() as sc_stack:
                sc = sc_stack.enter_context(tc.tile_pool(name="score", bufs=1))
                xh = sc.tile([P, KD, S], bf16)
                xl = sc.tile([P, KD, S], bf16)
                ch = sc.tile([P, KD, NCODE], bf16)
                cl = sc.tile([P, KD, NCODE], bf16)
                for k in range(KD):
                    nc.sync.dma_start(ch[:, k, :], fmv(cbnT_h, KD)[:, k, :])
                    nc.sync.dma_start(cl[:, k, :], fmv(cbnT_l, KD)[:, k, :])
                    nc.sync.dma_start(xh[:, k, :], fmv(featT_h, KD)[:, k, :])
                    nc.sync.dma_start(xl[:, k, :], fmv(featT_l, KD)[:, k, :])

                spool = sc_stack.enter_context(tc.tile_pool(name="svq", bufs=2))
                m8p = sc_stack.enter_context(tc.tile_pool(name="m8", bufs=2))

                for t in range(ST):
                    s_t = spool.tile([P, NCODE], f32, tag="svq")
                    for (n0, nl) in _chunks(NCODE, 512):
                        ps = mmp.tile([P, 512], f32, tag="mm")
                        cnt = 0
                        for (a, c) in ((xh, ch), (xh, cl), (xl, ch)):
                            for k in range(KD):
                                nc.tensor.matmul(
                                    ps[:, :nl],
                                    lhsT=a[:, k, t * P:(t + 1) * P],
                                    rhs=c[:, k, n0:n0 + nl],
                                    start=(cnt == 0), stop=(cnt == 23))
                                cnt += 1
                        nc.vector.tensor_copy(s_t[:, n0:n0 + nl], ps[:, :nl])
                    mx = m8p.tile([P, 8], f32, tag="mx")
                    ix = m8p.tile([P, 8], u16, tag="ix")
                    nc.vector.max(mx, s_t)
                    nc.vector.max_index(ix, mx, s_t)
                    v = 16 if t in (6, 13) else P   # pad slots keep index 0
                    nc.vector.tensor_copy(idx32[:v, t:t + 1], ix[:v, 0:1])

            dqp = sa_stack.enter_context(tc.tile_pool(name="dq", bufs=1))
            dq = dqp.tile([P, ST, D], f32)
            for t in range(ST):
                nc.gpsimd.indirect_dma_start(
                    out=dq[:, t, :], out_offset=None, in_=codebook,
                    in_offset=bass.IndirectOffsetOnAxis(
                        ap=idx32[:, t:t + 1], axis=0))

            # persistent residual stream (f32, feature-major, right side
            # so it doesn't interleave with the left-side stage stack)
            resid = top.enter_context(
                tc.tile_pool(name="resid", bufs=1, side="right"))
            tgt = resid.tile([P, KD, S], f32)

            # dq^T -> tgt (f32)
            for t in range(ST):
                for k in range(KD):
                    pt = mmp.tile([P, P], f32, tag="mm", name="ptf")
                    nc.tensor.transpose(pt, dq[:, t, k * P:(k + 1) * P], ident_f)
                    nc.vector.tensor_copy(tgt[:, k, t * P:(t + 1) * P], pt)

        # ---------- helpers ----------
        def layer_norm(g_col, b_col, x_out=None):
            """Per-448-token-chunk pipelined layernorm over d (partitions).
            Stats via ones-vector matmul on PE, rstd chain on [1,448] rows,
            broadcast on gpsimd, apply on DVE.  x_out: optional bf16 shadow."""
            with ExitStack() as ln_stack:
                rows = ln_stack.enter_context(tc.tile_pool(name="lnrows", bufs=2))
                cast = ln_stack.enter_context(tc.tile_pool(name="lncast", bufs=3))
                stp = ln_stack.enter_context(
                    tc.tile_pool(name="lnps", bufs=1, space="PSUM"))
                bc = ln_stack.enter_context(tc.tile_pool(name="lnbc", bufs=2))
                for (n0, nl) in _chunks(S, 448):
                    ps_s = stp.tile([1, 448], f32, tag="st_s", name="ps_s")
                    ps_q = stp.tile([1, 448], f32, tag="st_q", name="ps_q")
                    for k in range(KD):
                        xb = cast.tile([P, 448], bf16, tag="xb", name="xb")
                        nc.vector.tensor_copy(xb[:, :nl], tgt[:, k, n0:n0 + nl])
                        sq = cast.tile([P, 448], bf16, tag="sq", name="sq")
                        nc.scalar.activation(sq[:, :nl], xb[:, :nl], Act.Square)
                        nc.tensor.matmul(ps_s[:, :nl], lhsT=ones_col,
                                         rhs=xb[:, :nl],
                                         start=(k == 0), stop=(k == KD - 1))
                        nc.tensor.matmul(ps_q[:, :nl], lhsT=ones_col,
                                         rhs=sq[:, :nl],
                                         start=(k == 0), stop=(k == KD - 1))
                    mu = rows.tile([1, 448], f32, tag="mu", name="mu")
                    nc.vector.tensor_scalar_mul(mu[:, :nl], ps_s[:, :nl], 1.0 / D)
                    rst = rows.tile([1, 448], f32, tag="rst", name="rst")
                    # rst = meansq - mu^2 + eps -> sqrt -> reciprocal
                    nc.vector.tensor_scalar_mul(rst[:, :nl], ps_q[:, :nl], 1.0 / D)
                    msq = rows.tile([1, 448], f32, tag="msq", name="msq")
                    nc.vector.tensor_mul(msq[:, :nl], mu[:, :nl], mu[:, :nl])
                    nc.vector.tensor_tensor(rst[:, :nl], rst[:, :nl], msq[:, :nl],
                                            Alu.subtract)
                    nc.vector.tensor_scalar_add(rst[:, :nl], rst[:, :nl], EPS)
                    nc.scalar.activation(rst[:, :nl], rst[:, :nl], Act.Sqrt)
                    nc.vector.reciprocal(rst[:, :nl], rst[:, :nl])
                    mu_bc = bc.tile([P, 448], f32, tag="mubc", name="mu_bc")
                    nc.gpsimd.partition_broadcast(mu_bc[:, :nl], mu[:, :nl])
                    rs_bc = bc.tile([P, 448], f32, tag="rsbc", name="rs_bc")
                    nc.gpsimd.partition_broadcast(rs_bc[:, :nl], rst[:, :nl])
                    for k in range(KD):
                        xc = tgt[:, k, n0:n0 + nl]
                        nc.vector.tensor_tensor(xc, xc, mu_bc[:, :nl],
                                                Alu.subtract)
                        nc.vector.scalar_tensor_tensor(
                            xc, xc, g_col[:, k:k + 1], rs_bc[:, :nl],
                            Alu.mult, Alu.mult)
                        nc.vector.tensor_scalar_add(xc, xc, b_col[:, k:k + 1])
                        if x_out is not None:
                            nc.vector.tensor_copy(x_out[:, k, n0:n0 + nl], xc)

        def load_w(pool, dram_ap, nk, tag):
            t = pool.tile([P, nk, dram_ap.shape[-1]], bf16,
                          name=f"w_{tag}", tag=tag)
            nc.sync.dma_start(t[:], fmv(dram_ap, nk))
            return t

        def attention(wd, bd, v_from_tgt):
            """One MHA block + residual add into tgt.
            v_from_tgt: True -> k/v input is tgt (self-attn, k=q input);
                        False -> k/v input is feat (cross-attn; K/V are
                        LN-independent, so they run first and hide the
                        preceding layernorm's vector work)."""
            for b in range(BPC):
                off = b * SP
                with ExitStack() as batt:
                    bufp = batt.enter_context(tc.tile_pool(name="abuf", bufs=1))
                    qt = bufp.tile([P, KD, SP], bf16)
                    kt = bufp.tile([P, KD, SEQ], bf16)
                    vt = bufp.tile([P, SP // P, D], bf16)
                    at = bufp.tile([P, SP // P, SP], bf16)
                    ot = bufp.tile([P, KD, SEQ], bf16)
                    qk_b = bufp.tile([P, KD, SP], bf16)
                    srcp = batt.enter_context(tc.tile_pool(name="kvsrc", bufs=1))
                    vsrc = srcp.tile([P, KD, SP], bf16)

                    def build_qk():
                        with ExitStack() as post:
                            pospool = post.enter_context(
                                tc.tile_pool(name="pos", bufs=1))
                            pos_s = pospool.tile([P, KD, SEQ], bf16)
                            nc.sync.dma_start(pos_s[:], fmv(posT, KD))
                            for k in range(KD):
                                nc.vector.tensor_tensor(
                                    qk_b[:, k, :SEQ], tgt[:, k, off:off + SEQ],
                                    pos_s[:, k, :], Alu.add)
                                nc.vector.memset(qk_b[:, k, SEQ:], 0.0)

                    def proj_q(wq_s):
                        for (n0, nl) in _chunks(SP, 448):
                            for e in range(KD):
                                ps = mmp.tile([P, 512], f32, tag="mm")
                                for k in range(KD):
                                    nc.tensor.matmul(
                                        ps[:, :nl],
                                        lhsT=wq_s[:, k, e * P:(e + 1) * P],
                                        rhs=qk_b[:, k, n0:n0 + nl],
                                        start=(k == 0), stop=(k == KD - 1))
                                nc.vector.tensor_scalar_add(
                                    qt[:, e, n0:n0 + nl], ps[:, :nl],
                                    bd["bq"][:, e:e + 1])

                    def proj_k(wk_s, k_rhs):
                        for (n0, nl) in _chunks(SEQ, 392):
                            for e in range(KD):
                                ps = mmp.tile([P, 512], f32, tag="mm")
                                for k in range(KD):
                                    nc.tensor.matmul(
                                        ps[:, :nl],
                                        lhsT=wk_s[:, k, e * P:(e + 1) * P],
                                        rhs=k_rhs[:, k, n0:n0 + nl],
                                        start=(k == 0), stop=(k == KD - 1))
                                nc.vector.tensor_scalar_add(
                                    kt[:, e, n0:n0 + nl], ps[:, :nl],
                                    bd["bk"][:, e:e + 1])

                    def proj_v(wv_s):
                        for (n0, nl) in _chunks(D, 512):
                            for t in range(SP // P):
                                ps = mmp.tile([P, 512], f32, tag="mm")
                                for k in range(KD):
                                    nc.tensor.matmul(
                                        ps[:, :nl],
                                        lhsT=vsrc[:, k, t * P:(t + 1) * P],
                                        rhs=wv_s[:, k, n0:n0 + nl],
                                        start=(k == 0), stop=(k == KD - 1))
                                nc.vector.tensor_copy(
                                    vt[:, t, n0:n0 + nl], ps[:, :nl])

                    if v_from_tgt:
                        build_qk()
                        for k in range(KD):
                            nc.vector.tensor_copy(
                                vsrc[:, k, :], tgt[:, k, off:off + SP])
                        with ExitStack() as wst:
                            wp = wst.enter_context(
                                tc.tile_pool(name="wqk", bufs=1))
                            wq_s = load_w(wp, wd["wq"], KD, "wq")
                            wk_s = load_w(wp, wd["wk"], KD, "wk")
                            proj_q(wq_s)
                            proj_k(wk_s, qk_b)
                        with ExitStack() as wst:
                            wp = wst.enter_context(
                                tc.tile_pool(name="wv", bufs=1))
                            proj_v(load_w(wp, wd["wv"], KD, "wv"))
                    else:
                        # cross-attn: K/V from feat first (LN-independent)
                        nc.sync.dma_start(
                            vsrc[:], fmv(featT_h, KD)[:, :, off:off + SP])
                        with ExitStack() as wst:
                            wp = wst.enter_context(
                                tc.tile_pool(name="wkv", bufs=1))
                            wk_s = load_w(wp, wd["wk"], KD, "wk")
                            wv_s = load_w(wp, wd["wv"], KD, "wv")
                            proj_k(wk_s, vsrc)
                            proj_v(wv_s)
                        build_qk()
                        with ExitStack() as wst:
                            wp = wst.enter_context(
                                tc.tile_pool(name="wq", bufs=1))
                            proj_q(load_w(wp, wd["wq"], KD, "wq"))

                    # attention core
                    smp = batt.enter_context(tc.tile_pool(name="sm", bufs=2))
                    smallp = batt.enter_context(
                        tc.tile_pool(name="smsc", bufs=2))
                    for qi in range(SP // P):
                        s_ps = []
                        for (n0, nl) in _chunks(SEQ, 392):
                            ps = mmp.tile([P, 512], f32, tag="mm")
                            for e in range(KD):
                                nc.tensor.matmul(
                                    ps[:, :nl],
                                    lhsT=qt[:, e, qi * P:(qi + 1) * P],
                                    rhs=kt[:, e, n0:n0 + nl],
                                    start=(e == 0), stop=(e == KD - 1))
                            s_ps.append((ps, n0, nl))
                        # rowmax + exp straight from PSUM
                        mxs = []
                        for i, (ps, n0, nl) in enumerate(s_ps):
                            mx = smallp.tile([P, 1], f32, tag=f"mx{i}",
                                             name=f"mx{i}")
                            nc.vector.tensor_reduce(
                                mx, ps[:, :nl], mybir.AxisListType.X, Alu.max)
                            mxs.append(mx)
                        nb = smallp.tile([P, 1], f32, tag="nb", name="nb")
                        nc.vector.tensor_tensor(nb, mxs[0], mxs[1], Alu.max)
                        nc.vector.tensor_scalar_mul(nb, nb, -1.0 / 32.0)
                        a_t = smp.tile([P, SP], bf16, tag="arow")
                        rss = []
                        for i, (ps, n0, nl) in enumerate(s_ps):
                            rs = smallp.tile([P, 1], f32, tag=f"rs{i}",
                                             name=f"rs{i}")
                            nc.scalar.activation(a_t[:, n0:n0 + nl],
                                                 ps[:, :nl], Act.Exp,
                                                 bias=nb, scale=1.0 / 32.0,
                                                 accum_out=rs)
                            rss.append(rs)
                        nc.vector.memset(a_t[:, SEQ:], 0.0)
                        ri = smallp.tile([P, 1], f32, tag="ri", name="ri")
                        nc.vector.tensor_tensor(ri, rss[0], rss[1], Alu.add)
                        nc.vector.reciprocal(ri, ri)
                        nc.vector.tensor_scalar_mul(a_t[:, :SEQ],
                                                    a_t[:, :SEQ], ri)
                        for kc in range(SP // P):
                            pt = tpp.tile([P, P], bf16, tag="tpb")
                            nc.tensor.transpose(
                                pt, a_t[:, kc * P:(kc + 1) * P], ident_b)
                            nc.vector.tensor_copy(
                                at[:, kc, qi * P:(qi + 1) * P], pt)
                    # O^T = V^T A^T (+bv) [128, KD, SEQ]
                    for (n0, nl) in _chunks(SEQ, 392):
                        for e in range(KD):
                            ps = mmp.tile([P, 512], f32, tag="mm")
                            for kc in range(SP // P):
                                nc.tensor.matmul(
                                    ps[:, :nl],
                                    lhsT=vt[:, kc, e * P:(e + 1) * P],
                                    rhs=at[:, kc, n0:n0 + nl],
                                    start=(kc == 0), stop=(kc == SP // P - 1))
                            nc.vector.tensor_scalar_add(
                                ot[:, e, n0:n0 + nl], ps[:, :nl],
                                bd["bv"][:, e:e + 1])
                    # out-proj + bias + residual into tgt
                    with ExitStack() as wst:
                        wp = wst.enter_context(tc.tile_pool(name="wo", bufs=1))
                        wo_s = load_w(wp, wd["wo"], KD, "wo")
                        for (n0, nl) in _chunks(SEQ, 392):
                            for o in range(KD):
                                ps = mmp.tile([P, 512], f32, tag="mm")
                                for e in range(KD):
                                    nc.tensor.matmul(
                                        ps[:, :nl],
                                        lhsT=wo_s[:, e, o * P:(o + 1) * P],
                                        rhs=ot[:, e, n0:n0 + nl],
                                        start=(e == 0), stop=(e == KD - 1))
                                nc.vector.scalar_tensor_tensor(
                                    tgt[:, o, off + n0:off + n0 + nl],
                                    ps[:, :nl], bd["bo"][:, o:o + 1],
                                    tgt[:, o, off + n0:off + n0 + nl],
                                    Alu.add, Alu.add)

        # ================= decoder =================
        attention(w_sa, b_sa_s, v_from_tgt=True)
        layer_norm(ln_s["n1_g"], ln_s["n1_b"])
        attention(w_ca, b_ca_s, v_from_tgt=False)
        # x_h = bf16(tgt2), written chunk-wise by LN2
        xhp = top.enter_context(tc.tile_pool(name="xh2", bufs=1))
        x_h = xhp.tile([P, KD, S], bf16)
        layer_norm(ln_s["n2_g"], ln_s["n2_b"], x_out=x_h)

        # ---- FFN ----
        with ExitStack() as ffn:
            hp = ffn.enter_context(tc.tile_pool(name="hbuf", bufs=1))
            h_s = hp.tile([P, HK, S], bf16)
            with ExitStack() as w2l:
                w2p1 = w2l.enter_context(tc.tile_pool(name="w2a", bufs=1))
                w2a = w2p1.tile([P, KD, D], bf16)
                nc.sync.dma_start(w2a[:], fmv(lin2T, HK)[:, 0:KD, :])
                with ExitStack() as w1l:
                    w1p = w1l.enter_context(tc.tile_pool(name="w1", bufs=1))
                    w1 = load_w(w1p, lin1T, KD, "w1")
                    for (n0, nl) in _chunks(S, 448):
                        for ht in range(HK):
                            ps = mmp.tile([P, 512], f32, tag="mm")
                            for k in range(KD):
                                nc.tensor.matmul(
                                    ps[:, :nl],
                                    lhsT=w1[:, k, ht * P:(ht + 1) * P],
                                    rhs=x_h[:, k, n0:n0 + nl],
                                    start=(k == 0), stop=(k == KD - 1))
                            nc.scalar.activation(h_s[:, ht, n0:n0 + nl],
                                                 ps[:, :nl], Act.Relu,
                                                 bias=lin1_b_s[:, ht:ht + 1])
                w2p2 = w2l.enter_context(tc.tile_pool(name="w2b", bufs=1))
                w2b = w2p2.tile([P, KD, D], bf16)
                nc.sync.dma_start(w2b[:], fmv(lin2T, HK)[:, KD:HK, :])
                for (n0, nl) in _chunks(S, 448):
                    for o in range(KD):
                        ps = mmp.tile([P, 512], f32, tag="mm")
                        for hk in range(HK):
                            w2t = w2a if hk < KD else w2b
                            nc.tensor.matmul(
                                ps[:, :nl],
                                lhsT=w2t[:, hk % KD, o * P:(o + 1) * P],
                                rhs=h_s[:, hk, n0:n0 + nl],
                                start=(hk == 0), stop=(hk == HK - 1))
                        nc.vector.scalar_tensor_tensor(
                            tgt[:, o, n0:n0 + nl], ps[:, :nl],
                            lin2_b_s[:, o:o + 1], tgt[:, o, n0:n0 + nl],
                            Alu.add, Alu.add)

        layer_norm(ln_s["n3_g"], ln_s["n3_b"])

        # tgt += feat (stream f32 chunks)
        with ExitStack() as fr:
            fp = fr.enter_context(tc.tile_pool(name="fres", bufs=2))
            for k in range(KD):
                ft = fp.tile([P, S], f32, tag="ft")
                nc.sync.dma_start(ft[:], fmv(featT, KD)[:, k, :])
                nc.vector.tensor_tensor(tgt[:, k, :], tgt[:, k, :], ft[:],
                                        Alu.add)
        for k in range(KD):
            nc.vector.tensor_copy(x_h[:, k, :], tgt[:, k, :])

        # ---- heads ----
        with ExitStack() as hd:
            hdp = hd.enter_context(tc.tile_pool(name="heads", bufs=1))
            wpool = hd.enter_context(tc.tile_pool(name="headw", bufs=1))
            outp = hd.enter_context(tc.tile_pool(name="outs", bufs=3))
            b1T = hdp.tile([P, S], f32)
            f1_s = load_w(wpool, f1T, KD, "f1")
            for (n0, nl) in _chunks(S, 448):
                ps = mmp.tile([P, 512], f32, tag="mm")
                for k in range(KD):
                    nc.tensor.matmul(ps[:, :nl], lhsT=f1_s[:, k, :],
                                     rhs=x_h[:, k, n0:n0 + nl],
                                     start=(k == 0), stop=(k == KD - 1))
                nc.vector.tensor_scalar_add(b1T[:, n0:n0 + nl], ps[:, :nl],
                                            f1_b_s[:, 0:1])
            h2 = hdp.tile([P, KD, S], bf16)
            f2a_s = load_w(wpool, f2aT, KD, "f2a")
            for (n0, nl) in _chunks(S, 448):
                for e in range(KD):
                    ps = mmp.tile([P, 512], f32, tag="mm")
                    for k in range(KD):
                        nc.tensor.matmul(
                            ps[:, :nl], lhsT=f2a_s[:, k, e * P:(e + 1) * P],
                            rhs=x_h[:, k, n0:n0 + nl],
                            start=(k == 0), stop=(k == KD - 1))
                    nc.scalar.activation(h2[:, e, n0:n0 + nl], ps[:, :nl],
                                         Act.Relu, bias=f2a_b_s[:, e:e + 1])
            trT = hdp.tile([P, S], f32)
            f2b_s = load_w(wpool, f2bT, KD, "f2b")
            for (n0, nl) in _chunks(S, 448):
                ps = mmp.tile([P, 512], f32, tag="mm")
                for e in range(KD):
                    nc.tensor.matmul(ps[:, :nl], lhsT=f2b_s[:, e, :],
                                     rhs=h2[:, e, n0:n0 + nl],
                                     start=(e == 0), stop=(e == KD - 1))
                nc.vector.scalar_tensor_tensor(
                    trT[:, n0:n0 + nl], ps[:, :nl], f2b_b_s[:, 0:1],
                    b1T[:, n0:n0 + nl], Alu.add, Alu.add)

            # tr output (token-major via PE transpose)
            for t in range(ST):
                b, qi = divmod(t, SP // P)
                v = min(P, SEQ - qi * P)
                if v <= 0:
                    continue
                pt = mmp.tile([P, P], f32, tag="mm", name="ptf")
                nc.tensor.transpose(pt, trT[:, t * P:(t + 1) * P], ident_f)
                stg = outp.tile([P, RD], f32, tag="trstg")
                nc.vector.tensor_copy(stg[:], pt[:, :RD])
                nc.sync.dma_start(tr_out[b, qi * P:qi * P + v, :], stg[:v, :])

            # proj head
            trh = hdp.tile([P, S], bf16)
            nc.vector.tensor_copy(trh[:], trT[:])
            nc.vector.memset(trh[96:97, :], 1.0)
            ph_s = wpool.tile([P, PD], bf16, tag="ph")
            nc.sync.dma_start(ph_s[:], phT[:])
            for t in range(ST):
                b, qi = divmod(t, SP // P)
                v = min(P, SEQ - qi * P)
                if v <= 0:
                    continue
                for (n0, nl) in _chunks(PD, 512):
                    ps = mmp.tile([P, 512], f32, tag="mm")
                    nc.tensor.matmul(ps[:, :nl],
                                     lhsT=trh[:, t * P:(t + 1) * P],
                                     rhs=ph_s[:, n0:n0 + nl],
                                     start=True, stop=True)
                    stg = outp.tile([P, 512], f32, tag="pstg")
                    nc.vector.tensor_copy(stg[:, :nl], ps[:, :nl])
                    nc.sync.dma_start(
                        proj_out[b, qi * P:qi * P + v, n0:n0 + nl],
                        stg[:v, :nl])

    nc.compile()
    return nc


def _get_program():
    if "nc" not in _PROG:
        _PROG["nc"] = _build_program()
    return _PROG["nc"]


def _split_bf16(x):
    hi = x.astype(BF)
    lo = (x - hi.astype(np.float32)).astype(BF)
    return hi, lo


def _prep_host(inputs):
    """Build the per-core input maps (host-side prep/transposes)."""
    f = lambda k: np.ascontiguousarray(np.asarray(inputs[k], dtype=np.float32))
    feat = f("feat")                     # (16, 784, 1024)
    cb = f("codebook")                   # (2048, 1024)
    cbn = cb / np.linalg.norm(cb, axis=1, keepdims=True)
    cbnT = np.ascontiguousarray(cbn.T)   # (1024, 2048)
    cbnT_h, cbnT_l = _split_bf16(cbnT)

    posT = np.ascontiguousarray(f("query_pos").T).astype(BF)   # (1024, 784)

    def wT(w):
        return np.ascontiguousarray(np.asarray(w, np.float32).T).astype(BF)

    common = {
        "cbnT_h": cbnT_h, "cbnT_l": cbnT_l, "codebook": cb, "posT": posT,
        "lin1T": wT(inputs["lin1_w"]), "lin2T": wT(inputs["lin2_w"]),
        "f2aT": wT(inputs["f2a_w"]),
        "lin1_b": f("lin1_b"), "lin2_b": f("lin2_b"), "f2a_b": f("f2a_b"),
    }
    for pre in ("sa", "ca"):
        wi = f(pre + "_wi")
        bi = f(pre + "_bi")
        common[pre + "_wqT"] = wT(wi[:D])
        common[pre + "_wkT"] = wT(wi[D:2 * D])
        common[pre + "_wvT"] = wT(wi[2 * D:])
        common[pre + "_woT"] = wT(inputs[pre + "_wo"])
        common[pre + "_bq"] = bi[:D].copy()
        common[pre + "_bk"] = bi[D:2 * D].copy()
        common[pre + "_bv"] = bi[2 * D:].copy()
        common[pre + "_bo"] = f(pre + "_bo")
    # padded heads
    f1Tp = np.zeros((D, P), np.float32)
    f1Tp[:, :RD] = f("f1_w").T
    common["f1T"] = f1Tp.astype(BF)
    f2bTp = np.zeros((D, P), np.float32)
    f2bTp[:, :RD] = f("f2b_w").T
    common["f2bT"] = f2bTp.astype(BF)
    phTp = np.zeros((P, PD), np.float32)
    phTp[:RD] = f("ph_w").T
    phTp[96] = f("ph_b")
    common["phT"] = phTp.astype(BF)
    f1b = np.zeros((P,), np.float32)
    f1b[:RD] = f("f1_b")
    common["f1_b"] = f1b
    f2bb = np.zeros((P,), np.float32)
    f2bb[:RD] = f("f2b_b")
    common["f2b_b"] = f2bb
    for n in ("n1_g", "n1_b", "n2_g", "n2_b", "n3_g", "n3_b"):
        common[n] = f(n)

    in_maps = []
    for c in range(NCORES):
        fc = feat[c * BPC:(c + 1) * BPC]          # (2, 784, 1024)
        featT_c = np.zeros((D, S), np.float32)
        for b in range(BPC):
            featT_c[:, b * SP:b * SP + SEQ] = fc[b].T
        fh, fl = _split_bf16(featT_c)
        m = dict(common)
        m["featT"] = featT_c
        m["featT_h"] = fh
        m["featT_l"] = fl
        in_maps.append(m)
    return in_maps


def kernel(**inputs):
    _ensure_ntff_hook()
    from concourse.bass_utils import run_bass_kernel_spmd

    nc = _get_program()
    in_maps = _prep_host(inputs)
    res = run_bass_kernel_spmd(nc, in_maps, core_ids=list(range(NCORES)))
    tr = np.concatenate([res.results[c]["tr_out"] for c in range(NCORES)], 0)
    proj = np.concatenate([res.results[c]["proj_out"] for c in range(NCORES)], 0)
    return tr.astype(np.float32), proj.astype(np.float32)


# revision 22
# speedup vs baseline: 1.1211x; 1.0578x over previous
"""Trainium2 Bass kernel for nn_CAUSETRModel (VQ codebook + TR decoder).

Sharding: data-parallel over batch B=16 across 8 NeuronCores (2 batch
elements per core).  Everything else (codebook, decoder weights) is
replicated.  Inside each core the two batch elements are concatenated
along the token axis (2 x 896-padded = 1792 "slots", 784 real tokens
each).

Device dataflow (per core), feature-major activations [d on partitions,
tokens on free]:
  1. VQ scoring S = feat @ cbn^T as three bf16 matmuls (hi/lo split of
     both operands) so the cosine argmax matches fp32 exactly; row argmax
     via vector.max/max_index; codebook row gather via gpsimd.dma_gather.
  2. dq^T via PE transpose -> residual stream tgt (f32).
  3. Self-attn, cross-attn (per batch element), FFN, 3 layernorms
     (stats via ones-vector matmul on PE, broadcast via gpsimd),
     all matmuls in bf16 with f32 PSUM accumulation.
  4. Heads (f1 / f2a / f2b / proj) and token-major outputs via PE
     transpose.
"""

import sys
import types
from contextlib import ExitStack

for _p in ("/opt/trn_rl_repo", "/root/.axon_site"):
    if _p not in sys.path:
        sys.path.insert(0, _p)

import numpy as np
import ml_dtypes

BF = ml_dtypes.bfloat16

# ---- problem shapes (hardcoded) ----
B, SEQ, D = 16, 784, 1024
RD, PD, NCODE, HID = 90, 2048, 2048, 2048
EPS = 1e-5
NCORES = 8
BPC = B // NCORES          # batch elements per core = 2
SP = 896                   # per-batch padded token span (7*128)
S = BPC * SP               # 1792 concat padded tokens per core
ST = S // 128              # 14 token tiles
KD = D // 128              # 8 d-chunks
HK = HID // 128            # 16 hidden chunks
P = 128

_PROG = {}


def _ensure_ntff_hook():
    """Inject antenv.axon_hooks (absent in this image) so
    run_bass_kernel_spmd(trace=True) can profile via the axon .so."""
    import antenv
    if "antenv.axon_hooks" in sys.modules:
        return
    mod = types.ModuleType("antenv.axon_hooks")
    mod._hook = None
    mod.set_axon_ntff_profile_hook = lambda h: setattr(mod, "_hook", h)
    mod.get_axon_ntff_profile_hook = lambda: mod._hook
    sys.modules["antenv.axon_hooks"] = mod
    antenv.axon_hooks = mod
    try:
        from trn_agent_boot.trn_boot import _ntff_profile_via_ctypes
        mod.set_axon_ntff_profile_hook(
            _ntff_profile_via_ctypes("/opt/axon/libaxon_pjrt.so"))
    except Exception:
        pass


def _chunks(total, cap):
    out = []
    o = 0
    while o < total:
        c = min(cap, total - o)
        out.append((o, c))
        o += c
    return out


def _build_program():
    import concourse.bass as bass
    import concourse.tile as tile
    from concourse import bacc, mybir
    from concourse.masks import make_identity

    f32 = mybir.dt.float32
    bf16 = mybir.dt.bfloat16
    i16 = mybir.dt.int16
    u16 = mybir.dt.uint16
    Alu = mybir.AluOpType
    Act = mybir.ActivationFunctionType

    nc = bacc.Bacc("TRN2", target_bir_lowering=False, debug=False,
                   num_devices=NCORES)

    def din(name, shape, dt=f32):
        return nc.dram_tensor(name, list(shape), dt, kind="ExternalInput").ap()

    def dout(name, shape, dt=f32):
        return nc.dram_tensor(name, list(shape), dt, kind="ExternalOutput").ap()

    # ---- DRAM parameters ----
    featT = din("featT", (D, S))                  # f32, zero-padded cols
    featT_h = din("featT_h", (D, S), bf16)
    featT_l = din("featT_l", (D, S), bf16)
    cbnT_h = din("cbnT_h", (D, NCODE), bf16)
    cbnT_l = din("cbnT_l", (D, NCODE), bf16)
    codebook = din("codebook", (NCODE, D))
    posT = din("posT", (D, SEQ), bf16)

    w_sa = {n: din(f"sa_{n}T", (D, D), bf16) for n in ("wq", "wk", "wv", "wo")}
    w_ca = {n: din(f"ca_{n}T", (D, D), bf16) for n in ("wq", "wk", "wv", "wo")}
    lin1T = din("lin1T", (D, HID), bf16)
    lin2T = din("lin2T", (HID, D), bf16)
    f1T = din("f1T", (D, P), bf16)                # cols 90.. zero
    f2aT = din("f2aT", (D, D), bf16)
    f2bT = din("f2bT", (D, P), bf16)
    phT = din("phT", (P, PD), bf16)               # row 90 = ph_b, rows 91.. zero

    b_sa = {n: din(f"sa_{n}", (D,)) for n in ("bq", "bk", "bv", "bo")}
    b_ca = {n: din(f"ca_{n}", (D,)) for n in ("bq", "bk", "bv", "bo")}
    lin1_b = din("lin1_b", (HID,))
    lin2_b = din("lin2_b", (D,))
    f2a_b = din("f2a_b", (D,))
    f1_b = din("f1_b", (P,))                      # padded to 128
    f2b_b = din("f2b_b", (P,))
    ln_par = {n: din(n, (D,)) for n in
              ("n1_g", "n1_b", "n2_g", "n2_b", "n3_g", "n3_b")}

    tr_out = dout("tr_out", (BPC, SEQ, RD))
    proj_out = dout("proj_out", (BPC, SEQ, PD))

    # feature-major DRAM views
    def fmv(ap, nk):  # (nk*128, F) -> [128, nk, F]
        return ap.rearrange("(k p) f -> p k f", p=P)

    def colv(ap):     # (nk*128,) -> [128, nk]
        return ap.rearrange("(k p) -> p k", p=P)

    with tile.TileContext(nc) as tc, ExitStack() as top:
        const = top.enter_context(tc.tile_pool(name="const", bufs=1))
        ident_f = const.tile([P, P], f32)
        make_identity(nc, ident_f)
        ident_b = const.tile([P, P], bf16)
        make_identity(nc, ident_b)
        ones_col = const.tile([P, 1], bf16)
        nc.vector.memset(ones_col, 1.0)

        # all biases / ln params resident (tiny)
        bias = top.enter_context(tc.tile_pool(name="bias", bufs=1))

        def load_col(ap, nk, label):
            t = bias.tile([P, nk], f32, name=f"bc_{label}", tag=f"bc_{label}")
            nc.sync.dma_start(t[:], colv(ap))
            return t

        b_sa_s = {n: load_col(a, KD, f"sa{n}") for n, a in b_sa.items()}
        b_ca_s = {n: load_col(a, KD, f"ca{n}") for n, a in b_ca.items()}
        lin1_b_s = load_col(lin1_b, HK, "l1b")
        lin2_b_s = load_col(lin2_b, KD, "l2b")
        f2a_b_s = load_col(f2a_b, KD, "f2ab")
        f1_b_s = load_col(f1_b, 1, "f1b")
        f2b_b_s = load_col(f2b_b, 1, "f2bb")
        ln_s = {n: load_col(a, KD, n) for n, a in ln_par.items()}

        mmp = top.enter_context(tc.tile_pool(name="mmp", bufs=4, space="PSUM"))
        tpp = top.enter_context(tc.tile_pool(name="tpp", bufs=2, space="PSUM"))

        # ================= Stage A: VQ =================
        with ExitStack() as sa_stack:
            idxp = sa_stack.enter_context(tc.tile_pool(name="idxp", bufs=1))
            idx32 = idxp.tile([P, ST], mybir.dt.uint32)
            nc.vector.memset(idx32[:], 0)

            with ExitStack() as sc_stack:
                sc = sc_stack.enter_context(tc.tile_pool(name="score", bufs=1))
                xh = sc.tile([P, KD, S], bf16)
                xl = sc.tile([P, KD, S], bf16)
                ch = sc.tile([P, KD, NCODE], bf16)
                cl = sc.tile([P, KD, NCODE], bf16)
                for k in range(KD):
                    nc.sync.dma_start(ch[:, k, :], fmv(cbnT_h, KD)[:, k, :])
                    nc.sync.dma_start(xh[:, k, :], fmv(featT_h, KD)[:, k, :])
                for k in range(KD):
                    nc.sync.dma_start(cl[:, k, :], fmv(cbnT_l, KD)[:, k, :])
                    nc.sync.dma_start(xl[:, k, :], fmv(featT_l, KD)[:, k, :])

                spool = sc_stack.enter_context(tc.tile_pool(name="svq", bufs=2))
                m8p = sc_stack.enter_context(tc.tile_pool(name="m8", bufs=2))

                for t in range(ST):
                    s_t = spool.tile([P, NCODE], f32, tag="svq")
                    for (n0, nl) in _chunks(NCODE, 512):
                        ps = mmp.tile([P, 512], f32, tag="mm")
                        cnt = 0
                        for (a, c) in ((xh, ch), (xh, cl), (xl, ch)):
                            for k in range(KD):
                                nc.tensor.matmul(
                                    ps[:, :nl],
                                    lhsT=a[:, k, t * P:(t + 1) * P],
                                    rhs=c[:, k, n0:n0 + nl],
                                    start=(cnt == 0), stop=(cnt == 23))
                                cnt += 1
                        nc.vector.tensor_copy(s_t[:, n0:n0 + nl], ps[:, :nl])
                    mx = m8p.tile([P, 8], f32, tag="mx")
                    ix = m8p.tile([P, 8], u16, tag="ix")
                    nc.vector.max(mx, s_t)
                    nc.vector.max_index(ix, mx, s_t)
                    v = 16 if t in (6, 13) else P   # pad slots keep index 0
                    nc.vector.tensor_copy(idx32[:v, t:t + 1], ix[:v, 0:1])

            dqp = sa_stack.enter_context(tc.tile_pool(name="dq", bufs=1))
            dq = dqp.tile([P, ST, D], f32)
            for t in range(ST):
                nc.gpsimd.indirect_dma_start(
                    out=dq[:, t, :], out_offset=None, in_=codebook,
                    in_offset=bass.IndirectOffsetOnAxis(
                        ap=idx32[:, t:t + 1], axis=0))

            # persistent residual stream (f32, feature-major, right side
            # so it doesn't interleave with the left-side stage stack)
            resid = top.enter_context(
                tc.tile_pool(name="resid", bufs=1, side="right"))
            tgt = resid.tile([P, KD, S], f32)

            # dq^T -> tgt (f32)
            for t in range(ST):
                for k in range(KD):
                    pt = mmp.tile([P, P], f32, tag="mm", name="ptf")
                    nc.tensor.transpose(pt, dq[:, t, k * P:(k + 1) * P], ident_f)
                    nc.vector.tensor_copy(tgt[:, k, t * P:(t + 1) * P], pt)

        # ---------- helpers ----------
        def layer_norm(g_col, b_col, x_out=None):
            """Per-448-token-chunk pipelined layernorm over d (partitions).
            Stats via ones-vector matmul on PE, rstd chain on [1,448] rows,
            broadcast on gpsimd, apply on DVE.  x_out: optional bf16 shadow."""
            with ExitStack() as ln_stack:
                rows = ln_stack.enter_context(tc.tile_pool(name="lnrows", bufs=2, side="right"))
                cast = ln_stack.enter_context(tc.tile_pool(name="lncast", bufs=3, side="right"))
                stp = ln_stack.enter_context(
                    tc.tile_pool(name="lnps", bufs=1, space="PSUM"))
                bc = ln_stack.enter_context(tc.tile_pool(name="lnbc", bufs=2, side="right"))
                for (n0, nl) in _chunks(S, 448):
                    ps_s = stp.tile([1, 448], f32, tag="st_s", name="ps_s")
                    ps_q = stp.tile([1, 448], f32, tag="st_q", name="ps_q")
                    for k in range(KD):
                        xb = cast.tile([P, 448], bf16, tag="xb", name="xb")
                        nc.vector.tensor_copy(xb[:, :nl], tgt[:, k, n0:n0 + nl])
                        sq = cast.tile([P, 448], bf16, tag="sq", name="sq")
                        nc.scalar.activation(sq[:, :nl], xb[:, :nl], Act.Square)
                        nc.tensor.matmul(ps_s[:, :nl], lhsT=ones_col,
                                         rhs=xb[:, :nl],
                                         start=(k == 0), stop=(k == KD - 1))
                        nc.tensor.matmul(ps_q[:, :nl], lhsT=ones_col,
                                         rhs=sq[:, :nl],
                                         start=(k == 0), stop=(k == KD - 1))
                    mu = rows.tile([1, 448], f32, tag="mu", name="mu")
                    nc.vector.tensor_scalar_mul(mu[:, :nl], ps_s[:, :nl], 1.0 / D)
                    rst = rows.tile([1, 448], f32, tag="rst", name="rst")
                    # rst = meansq - mu^2 + eps -> sqrt -> reciprocal
                    nc.vector.tensor_scalar_mul(rst[:, :nl], ps_q[:, :nl], 1.0 / D)
                    msq = rows.tile([1, 448], f32, tag="msq", name="msq")
                    nc.vector.tensor_mul(msq[:, :nl], mu[:, :nl], mu[:, :nl])
                    nc.vector.tensor_tensor(rst[:, :nl], rst[:, :nl], msq[:, :nl],
                                            Alu.subtract)
                    nc.vector.tensor_scalar_add(rst[:, :nl], rst[:, :nl], EPS)
                    nc.scalar.activation(rst[:, :nl], rst[:, :nl], Act.Sqrt)
                    nc.vector.reciprocal(rst[:, :nl], rst[:, :nl])
                    mu_bc = bc.tile([P, 448], f32, tag="mubc", name="mu_bc")
                    nc.gpsimd.partition_broadcast(mu_bc[:, :nl], mu[:, :nl])
                    rs_bc = bc.tile([P, 448], f32, tag="rsbc", name="rs_bc")
                    nc.gpsimd.partition_broadcast(rs_bc[:, :nl], rst[:, :nl])
                    for k in range(KD):
                        xc = tgt[:, k, n0:n0 + nl]
                        nc.vector.tensor_tensor(xc, xc, mu_bc[:, :nl],
                                                Alu.subtract)
                        nc.vector.scalar_tensor_tensor(
                            xc, xc, g_col[:, k:k + 1], rs_bc[:, :nl],
                            Alu.mult, Alu.mult)
                        nc.vector.tensor_scalar_add(xc, xc, b_col[:, k:k + 1])
                        if x_out is not None:
                            nc.vector.tensor_copy(x_out[:, k, n0:n0 + nl], xc)

        def load_w(pool, dram_ap, nk, tag):
            t = pool.tile([P, nk, dram_ap.shape[-1]], bf16,
                          name=f"w_{tag}", tag=tag)
            nc.sync.dma_start(t[:], fmv(dram_ap, nk))
            return t

        def attention(wd, bd, v_from_tgt):
            """One MHA block + residual add into tgt.
            v_from_tgt: True -> k/v input is tgt (self-attn, k=q input);
                        False -> k/v input is feat (cross-attn; K/V are
                        LN-independent, so they run first and hide the
                        preceding layernorm's vector work)."""
            for b in range(BPC):
                off = b * SP
                with ExitStack() as batt:
                    kvp = batt.enter_context(tc.tile_pool(name="kvbuf", bufs=1))
                    kt = kvp.tile([P, KD, SEQ], bf16)
                    vt = kvp.tile([P, SP // P, D], bf16)
                    vsrc = kvp.tile([P, KD, SP], bf16)

                    def build_qk(qk_b):
                        with ExitStack() as post:
                            pospool = post.enter_context(
                                tc.tile_pool(name="pos", bufs=1))
                            pos_s = pospool.tile([P, KD, SEQ], bf16)
                            nc.sync.dma_start(pos_s[:], fmv(posT, KD))
                            for k in range(KD):
                                nc.vector.tensor_tensor(
                                    qk_b[:, k, :SEQ], tgt[:, k, off:off + SEQ],
                                    pos_s[:, k, :], Alu.add)
                                nc.vector.memset(qk_b[:, k, SEQ:], 0.0)

                    def proj_q(wq_s, qt, qk_b):
                        for (n0, nl) in _chunks(SP, 448):
                            for e in range(KD):
                                ps = mmp.tile([P, 512], f32, tag="mm")
                                for k in range(KD):
                                    nc.tensor.matmul(
                                        ps[:, :nl],
                                        lhsT=wq_s[:, k, e * P:(e + 1) * P],
                                        rhs=qk_b[:, k, n0:n0 + nl],
                                        start=(k == 0), stop=(k == KD - 1))
                                nc.vector.tensor_scalar_add(
                                    qt[:, e, n0:n0 + nl], ps[:, :nl],
                                    bd["bq"][:, e:e + 1])

                    def proj_k(wk_s, k_rhs):
                        for (n0, nl) in _chunks(SEQ, 392):
                            for e in range(KD):
                                ps = mmp.tile([P, 512], f32, tag="mm")
                                for k in range(KD):
                                    nc.tensor.matmul(
                                        ps[:, :nl],
                                        lhsT=wk_s[:, k, e * P:(e + 1) * P],
                                        rhs=k_rhs[:, k, n0:n0 + nl],
                                        start=(k == 0), stop=(k == KD - 1))
                                nc.vector.tensor_scalar_add(
                                    kt[:, e, n0:n0 + nl], ps[:, :nl],
                                    bd["bk"][:, e:e + 1])

                    def proj_v(wv_s):
                        for (n0, nl) in _chunks(D, 512):
                            for t in range(SP // P):
                                ps = mmp.tile([P, 512], f32, tag="mm")
                                for k in range(KD):
                                    nc.tensor.matmul(
                                        ps[:, :nl],
                                        lhsT=vsrc[:, k, t * P:(t + 1) * P],
                                        rhs=wv_s[:, k, n0:n0 + nl],
                                        start=(k == 0), stop=(k == KD - 1))
                                nc.vector.tensor_copy(
                                    vt[:, t, n0:n0 + nl], ps[:, :nl])

                    def open_qpool():
                        bufp = batt.enter_context(
                            tc.tile_pool(name="abuf", bufs=1))
                        return (bufp.tile([P, KD, SP], bf16, name="qt"),
                                bufp.tile([P, SP // P, SP], bf16, name="at"),
                                bufp.tile([P, KD, SEQ], bf16, name="ot"),
                                bufp.tile([P, KD, SP], bf16, name="qk_b"))

                    if v_from_tgt:
                        qt, at, ot, qk_b = open_qpool()
                        build_qk(qk_b)
                        for k in range(KD):
                            nc.vector.tensor_copy(
                                vsrc[:, k, :], tgt[:, k, off:off + SP])
                        with ExitStack() as wst:
                            wp = wst.enter_context(
                                tc.tile_pool(name="wqk", bufs=1))
                            wq_s = load_w(wp, wd["wq"], KD, "wq")
                            wk_s = load_w(wp, wd["wk"], KD, "wk")
                            proj_q(wq_s, qt, qk_b)
                            proj_k(wk_s, qk_b)
                        with ExitStack() as wst:
                            wp = wst.enter_context(
                                tc.tile_pool(name="wv", bufs=1))
                            proj_v(load_w(wp, wd["wv"], KD, "wv"))
                    else:
                        # cross-attn: K/V from feat first (LN-independent)
                        nc.sync.dma_start(
                            vsrc[:], fmv(featT_h, KD)[:, :, off:off + SP])
                        with ExitStack() as wst:
                            wp = wst.enter_context(
                                tc.tile_pool(name="wkv", bufs=1))
                            wk_s = load_w(wp, wd["wk"], KD, "wk")
                            wv_s = load_w(wp, wd["wv"], KD, "wv")
                            proj_k(wk_s, vsrc)
                            proj_v(wv_s)
                        qt, at, ot, qk_b = open_qpool()
                        build_qk(qk_b)
                        with ExitStack() as wst:
                            wp = wst.enter_context(
                                tc.tile_pool(name="wq", bufs=1))
                            proj_q(load_w(wp, wd["wq"], KD, "wq"), qt, qk_b)

                    # attention core
                    smp = batt.enter_context(tc.tile_pool(name="sm", bufs=2))
                    smallp = batt.enter_context(
                        tc.tile_pool(name="smsc", bufs=2))
                    for qi in range(SP // P):
                        s_ps = []
                        for (n0, nl) in _chunks(SEQ, 392):
                            ps = mmp.tile([P, 512], f32, tag="mm")
                            for e in range(KD):
                                nc.tensor.matmul(
                                    ps[:, :nl],
                                    lhsT=qt[:, e, qi * P:(qi + 1) * P],
                                    rhs=kt[:, e, n0:n0 + nl],
                                    start=(e == 0), stop=(e == KD - 1))
                            s_ps.append((ps, n0, nl))
                        # rowmax + exp straight from PSUM
                        mxs = []
                        for i, (ps, n0, nl) in enumerate(s_ps):
                            mx = smallp.tile([P, 1], f32, tag=f"mx{i}",
                                             name=f"mx{i}")
                            nc.vector.tensor_reduce(
                                mx, ps[:, :nl], mybir.AxisListType.X, Alu.max)
                            mxs.append(mx)
                        nb = smallp.tile([P, 1], f32, tag="nb", name="nb")
                        nc.vector.tensor_tensor(nb, mxs[0], mxs[1], Alu.max)
                        nc.vector.tensor_scalar_mul(nb, nb, -1.0 / 32.0)
                        a_t = smp.tile([P, SP], bf16, tag="arow")
                        rss = []
                        for i, (ps, n0, nl) in enumerate(s_ps):
                            rs = smallp.tile([P, 1], f32, tag=f"rs{i}",
                                             name=f"rs{i}")
                            nc.scalar.activation(a_t[:, n0:n0 + nl],
                                                 ps[:, :nl], Act.Exp,
                                                 bias=nb, scale=1.0 / 32.0,
                                                 accum_out=rs)
                            rss.append(rs)
                        nc.vector.memset(a_t[:, SEQ:], 0.0)
                        ri = smallp.tile([P, 1], f32, tag="ri", name="ri")
                        nc.vector.tensor_tensor(ri, rss[0], rss[1], Alu.add)
                        nc.vector.reciprocal(ri, ri)
                        nc.vector.tensor_scalar_mul(a_t[:, :SEQ],
                                                    a_t[:, :SEQ], ri)
                        for kc in range(SP // P):
                            pt = tpp.tile([P, P], bf16, tag="tpb")
                            nc.tensor.transpose(
                                pt, a_t[:, kc * P:(kc + 1) * P], ident_b)
                            nc.vector.tensor_copy(
                                at[:, kc, qi * P:(qi + 1) * P], pt)
                    # O^T = V^T A^T (+bv) [128, KD, SEQ]
                    for (n0, nl) in _chunks(SEQ, 392):
                        for e in range(KD):
                            ps = mmp.tile([P, 512], f32, tag="mm")
                            for kc in range(SP // P):
                                nc.tensor.matmul(
                                    ps[:, :nl],
                                    lhsT=vt[:, kc, e * P:(e + 1) * P],
                                    rhs=at[:, kc, n0:n0 + nl],
                                    start=(kc == 0), stop=(kc == SP // P - 1))
                            nc.vector.tensor_scalar_add(
                                ot[:, e, n0:n0 + nl], ps[:, :nl],
                                bd["bv"][:, e:e + 1])
                    # out-proj + bias + residual into tgt
                    with ExitStack() as wst:
                        wp = wst.enter_context(tc.tile_pool(name="wo", bufs=1))
                        wo_s = load_w(wp, wd["wo"], KD, "wo")
                        for (n0, nl) in _chunks(SEQ, 392):
                            for o in range(KD):
                                ps = mmp.tile([P, 512], f32, tag="mm")
                                for e in range(KD):
                                    nc.tensor.matmul(
                                        ps[:, :nl],
                                        lhsT=wo_s[:, e, o * P:(o + 1) * P],
                                        rhs=ot[:, e, n0:n0 + nl],
                                        start=(e == 0), stop=(e == KD - 1))
                                nc.vector.scalar_tensor_tensor(
                                    tgt[:, o, off + n0:off + n0 + nl],
                                    ps[:, :nl], bd["bo"][:, o:o + 1],
                                    tgt[:, o, off + n0:off + n0 + nl],
                                    Alu.add, Alu.add)

        # ================= decoder =================
        attention(w_sa, b_sa_s, v_from_tgt=True)
        layer_norm(ln_s["n1_g"], ln_s["n1_b"])
        attention(w_ca, b_ca_s, v_from_tgt=False)
        # x_h = bf16(tgt2), written chunk-wise by LN2
        xhp = top.enter_context(tc.tile_pool(name="xh2", bufs=1, side="right"))
        x_h = xhp.tile([P, KD, S], bf16)
        layer_norm(ln_s["n2_g"], ln_s["n2_b"], x_out=x_h)

        # ---- FFN ----
        with ExitStack() as ffn:
            hp = ffn.enter_context(tc.tile_pool(name="hbuf", bufs=1))
            h_s = hp.tile([P, HK, S], bf16)
            with ExitStack() as w2l:
                w2p1 = w2l.enter_context(tc.tile_pool(name="w2a", bufs=1))
                w2a = w2p1.tile([P, KD, D], bf16)
                nc.sync.dma_start(w2a[:], fmv(lin2T, HK)[:, 0:KD, :])
                with ExitStack() as w1l:
                    w1p = w1l.enter_context(tc.tile_pool(name="w1", bufs=1))
                    w1 = load_w(w1p, lin1T, KD, "w1")
                    for (n0, nl) in _chunks(S, 448):
                        for ht in range(HK):
                            ps = mmp.tile([P, 512], f32, tag="mm")
                            for k in range(KD):
                                nc.tensor.matmul(
                                    ps[:, :nl],
                                    lhsT=w1[:, k, ht * P:(ht + 1) * P],
                                    rhs=x_h[:, k, n0:n0 + nl],
                                    start=(k == 0), stop=(k == KD - 1))
                            nc.scalar.activation(h_s[:, ht, n0:n0 + nl],
                                                 ps[:, :nl], Act.Relu,
                                                 bias=lin1_b_s[:, ht:ht + 1])
                w2p2 = w2l.enter_context(tc.tile_pool(name="w2b", bufs=1))
                w2b = w2p2.tile([P, KD, D], bf16)
                nc.sync.dma_start(w2b[:], fmv(lin2T, HK)[:, KD:HK, :])
                for (n0, nl) in _chunks(S, 448):
                    for o in range(KD):
                        ps = mmp.tile([P, 512], f32, tag="mm")
                        for hk in range(HK):
                            w2t = w2a if hk < KD else w2b
                            nc.tensor.matmul(
                                ps[:, :nl],
                                lhsT=w2t[:, hk % KD, o * P:(o + 1) * P],
                                rhs=h_s[:, hk, n0:n0 + nl],
                                start=(hk == 0), stop=(hk == HK - 1))
                        nc.vector.scalar_tensor_tensor(
                            tgt[:, o, n0:n0 + nl], ps[:, :nl],
                            lin2_b_s[:, o:o + 1], tgt[:, o, n0:n0 + nl],
                            Alu.add, Alu.add)

        layer_norm(ln_s["n3_g"], ln_s["n3_b"])

        # tgt += feat (stream f32 chunks)
        with ExitStack() as fr:
            fp = fr.enter_context(tc.tile_pool(name="fres", bufs=2))
            for k in range(KD):
                ft = fp.tile([P, S], f32, tag="ft")
                nc.sync.dma_start(ft[:], fmv(featT, KD)[:, k, :])
                nc.vector.tensor_tensor(tgt[:, k, :], tgt[:, k, :], ft[:],
                                        Alu.add)
        for k in range(KD):
            nc.vector.tensor_copy(x_h[:, k, :], tgt[:, k, :])

        # ---- heads ----
        with ExitStack() as hd:
            hdp = hd.enter_context(tc.tile_pool(name="heads", bufs=1))
            wpool = hd.enter_context(tc.tile_pool(name="headw", bufs=1))
            outp = hd.enter_context(tc.tile_pool(name="outs", bufs=3))
            b1T = hdp.tile([P, S], f32)
            f1_s = load_w(wpool, f1T, KD, "f1")
            for (n0, nl) in _chunks(S, 448):
                ps = mmp.tile([P, 512], f32, tag="mm")
                for k in range(KD):
                    nc.tensor.matmul(ps[:, :nl], lhsT=f1_s[:, k, :],
                                     rhs=x_h[:, k, n0:n0 + nl],
                                     start=(k == 0), stop=(k == KD - 1))
                nc.vector.tensor_scalar_add(b1T[:, n0:n0 + nl], ps[:, :nl],
                                            f1_b_s[:, 0:1])
            h2 = hdp.tile([P, KD, S], bf16)
            f2a_s = load_w(wpool, f2aT, KD, "f2a")
            for (n0, nl) in _chunks(S, 448):
                for e in range(KD):
                    ps = mmp.tile([P, 512], f32, tag="mm")
                    for k in range(KD):
                        nc.tensor.matmul(
                            ps[:, :nl], lhsT=f2a_s[:, k, e * P:(e + 1) * P],
                            rhs=x_h[:, k, n0:n0 + nl],
                            start=(k == 0), stop=(k == KD - 1))
                    nc.scalar.activation(h2[:, e, n0:n0 + nl], ps[:, :nl],
                                         Act.Relu, bias=f2a_b_s[:, e:e + 1])
            trT = hdp.tile([P, S], f32)
            f2b_s = load_w(wpool, f2bT, KD, "f2b")
            for (n0, nl) in _chunks(S, 448):
                ps = mmp.tile([P, 512], f32, tag="mm")
                for e in range(KD):
                    nc.tensor.matmul(ps[:, :nl], lhsT=f2b_s[:, e, :],
                                     rhs=h2[:, e, n0:n0 + nl],
                                     start=(e == 0), stop=(e == KD - 1))
                nc.vector.scalar_tensor_tensor(
                    trT[:, n0:n0 + nl], ps[:, :nl], f2b_b_s[:, 0:1],
                    b1T[:, n0:n0 + nl], Alu.add, Alu.add)

            # tr output (token-major via PE transpose)
            for t in range(ST):
                b, qi = divmod(t, SP // P)
                v = min(P, SEQ - qi * P)
                if v <= 0:
                    continue
                pt = mmp.tile([P, P], f32, tag="mm", name="ptf")
                nc.tensor.transpose(pt, trT[:, t * P:(t + 1) * P], ident_f)
                stg = outp.tile([P, RD], f32, tag="trstg")
                nc.vector.tensor_copy(stg[:], pt[:, :RD])
                nc.sync.dma_start(tr_out[b, qi * P:qi * P + v, :], stg[:v, :])

            # proj head
            trh = hdp.tile([P, S], bf16)
            nc.vector.tensor_copy(trh[:], trT[:])
            nc.vector.memset(trh[96:97, :], 1.0)
            ph_s = wpool.tile([P, PD], bf16, tag="ph")
            nc.sync.dma_start(ph_s[:], phT[:])
            for t in range(ST):
                b, qi = divmod(t, SP // P)
                v = min(P, SEQ - qi * P)
                if v <= 0:
                    continue
                for (n0, nl) in _chunks(PD, 512):
                    ps = mmp.tile([P, 512], f32, tag="mm")
                    nc.tensor.matmul(ps[:, :nl],
                                     lhsT=trh[:, t * P:(t + 1) * P],
                                     rhs=ph_s[:, n0:n0 + nl],
                                     start=True, stop=True)
                    stg = outp.tile([P, 512], f32, tag="pstg")
                    nc.vector.tensor_copy(stg[:, :nl], ps[:, :nl])
                    nc.sync.dma_start(
                        proj_out[b, qi * P:qi * P + v, n0:n0 + nl],
                        stg[:v, :nl])

    nc.compile()
    return nc


def _get_program():
    if "nc" not in _PROG:
        _PROG["nc"] = _build_program()
    return _PROG["nc"]


def _split_bf16(x):
    hi = x.astype(BF)
    lo = (x - hi.astype(np.float32)).astype(BF)
    return hi, lo


def _prep_host(inputs):
    """Build the per-core input maps (host-side prep/transposes)."""
    f = lambda k: np.ascontiguousarray(np.asarray(inputs[k], dtype=np.float32))
    feat = f("feat")                     # (16, 784, 1024)
    cb = f("codebook")                   # (2048, 1024)
    cbn = cb / np.linalg.norm(cb, axis=1, keepdims=True)
    cbnT = np.ascontiguousarray(cbn.T)   # (1024, 2048)
    cbnT_h, cbnT_l = _split_bf16(cbnT)

    posT = np.ascontiguousarray(f("query_pos").T).astype(BF)   # (1024, 784)

    def wT(w):
        return np.ascontiguousarray(np.asarray(w, np.float32).T).astype(BF)

    common = {
        "cbnT_h": cbnT_h, "cbnT_l": cbnT_l, "codebook": cb, "posT": posT,
        "lin1T": wT(inputs["lin1_w"]), "lin2T": wT(inputs["lin2_w"]),
        "f2aT": wT(inputs["f2a_w"]),
        "lin1_b": f("lin1_b"), "lin2_b": f("lin2_b"), "f2a_b": f("f2a_b"),
    }
    for pre in ("sa", "ca"):
        wi = f(pre + "_wi")
        bi = f(pre + "_bi")
        common[pre + "_wqT"] = wT(wi[:D])
        common[pre + "_wkT"] = wT(wi[D:2 * D])
        common[pre + "_wvT"] = wT(wi[2 * D:])
        common[pre + "_woT"] = wT(inputs[pre + "_wo"])
        common[pre + "_bq"] = bi[:D].copy()
        common[pre + "_bk"] = bi[D:2 * D].copy()
        common[pre + "_bv"] = bi[2 * D:].copy()
        common[pre + "_bo"] = f(pre + "_bo")
    # padded heads
    f1Tp = np.zeros((D, P), np.float32)
    f1Tp[:, :RD] = f("f1_w").T
    common["f1T"] = f1Tp.astype(BF)
    f2bTp = np.zeros((D, P), np.float32)
    f2bTp[:, :RD] = f("f2b_w").T
    common["f2bT"] = f2bTp.astype(BF)
    phTp = np.zeros((P, PD), np.float32)
    phTp[:RD] = f("ph_w").T
    phTp[96] = f("ph_b")
    common["phT"] = phTp.astype(BF)
    f1b = np.zeros((P,), np.float32)
    f1b[:RD] = f("f1_b")
    common["f1_b"] = f1b
    f2bb = np.zeros((P,), np.float32)
    f2bb[:RD] = f("f2b_b")
    common["f2b_b"] = f2bb
    for n in ("n1_g", "n1_b", "n2_g", "n2_b", "n3_g", "n3_b"):
        common[n] = f(n)

    in_maps = []
    for c in range(NCORES):
        fc = feat[c * BPC:(c + 1) * BPC]          # (2, 784, 1024)
        featT_c = np.zeros((D, S), np.float32)
        for b in range(BPC):
            featT_c[:, b * SP:b * SP + SEQ] = fc[b].T
        fh, fl = _split_bf16(featT_c)
        m = dict(common)
        m["featT"] = featT_c
        m["featT_h"] = fh
        m["featT_l"] = fl
        in_maps.append(m)
    return in_maps


def kernel(**inputs):
    _ensure_ntff_hook()
    from concourse.bass_utils import run_bass_kernel_spmd

    nc = _get_program()
    in_maps = _prep_host(inputs)
    res = run_bass_kernel_spmd(nc, in_maps, core_ids=list(range(NCORES)))
    tr = np.concatenate([res.results[c]["tr_out"] for c in range(NCORES)], 0)
    proj = np.concatenate([res.results[c]["proj_out"] for c in range(NCORES)], 0)
    return tr.astype(np.float32), proj.astype(np.float32)
